# revision 32
# baseline (speedup 1.0000x reference)
"""MoE expert-parallel kernel for Trainium2 (8 NeuronCores).

Strategy:
  - Host: route tokens to experts (stable sort by dispatch_order). Experts are
    assigned to (core, slot) pairs by descending token count: slot j of core c
    gets the (8*j + c)-th most-loaded expert, so all cores see nearly identical
    work and slot j's capacity cap_j = max over cores of its count (tight).
  - Device (SPMD, 8 cores, 8 expert slots/core):
    per slot: HT = gelu(w1^T-tiled @ XT + b1) computed transposed [F, tokens],
    then Y = HT^T @ w2 + b2 [tokens, D]; bf16 operands, fp32 PSUM accumulation.
  - Host: scatter per-expert outputs back to original token order.

Startup critical path: the first matmul needs only slot-0's xt and the first
f-tiles of slot-0's w1. Slot-0 w1 is stored f-tile-major ([128, KF, KD, 128])
so progressive f-blocks are contiguous per partition, and the early blocks go
on the Scalar HWDGE ring while xt chunks go on the Sync ring — the two DGEs
generate descriptors in parallel and neither queues behind the other.

Exit critical path: the final tile's y rows are split across the Sync and
Scalar HWDGE rings (both stripe packets over all 16 SDMA engines); the GpSimd
software queue is avoided (it lumps a whole transfer onto one engine).

No cross-core collectives: each core owns a disjoint set of experts, hence a
disjoint set of output token rows.
"""

import sys

import numpy as np
import ml_dtypes

for _p in ("/opt/trn_rl_repo",):
    if _p not in sys.path:
        sys.path.insert(0, _p)

_BF16 = ml_dtypes.bfloat16
_F8 = getattr(ml_dtypes, "float8_e4m3", ml_dtypes.float8_e4m3fn)

NUM_EXPERTS = 64
N_CORES = 8
E_LOCAL = NUM_EXPERTS // N_CORES  # 8 expert slots per core
D = 512
F = 2048
KD = D // 128   # 4 contraction tiles for layer 1
KF = F // 128   # 16 contraction tiles for layer 2

# Layer-2 k-tiles 0-1 run as one fp8 DoubleRow matmul (2x PE rate). The fp8
# quantization error on 1/8 of the contraction keeps the end-to-end max
# relative error at ~1.6e-2 (vs 3.5e-3 pure-bf16), under the 2e-2 budget.
# w2's fp8 copy is pre-scaled by _W28_SCALE (its values ~0.02 would land in
# e4m3's denormal range unscaled); the partial sum is descaled in the DVE
# epilogue, which is why it accumulates in a separate PSUM tile.
FP8_KT = 2
_W28_SCALE = 64.0

_nc_cache = {}


def _chunk_list(cap, e):
    """Layer-1 token chunks per slot (PSUM free dim <= 512 fp32).

    Balanced halves for cap > 512: a tiny trailing chunk would pay a full
    weight-load pass for a handful of columns.
    """
    if cap == 0:
        return []
    if cap <= 512:
        return [cap]
    h = (cap + 1) // 2
    return [h, cap - h]


def _slot_geometry(caps):
    """Per-slot column offsets for xt and row offsets for y."""
    xoff = [0]
    yoff = [0]
    for c in caps:
        xoff.append(xoff[-1] + c)
        yoff.append(yoff[-1] + (-(-c // 128)) * 128)
    return xoff, yoff


# Slot-0 startup: the first xt chunk and w1 f-tiles 0-1 are fused into one
# "boot" transfer (~4.3 KB per-partition descriptors — big descriptors are
# what the SDMA engines sustain high rates on) issued first on the Sync
# ring, so a single early completion unblocks the first matmuls. Later
# f-blocks: [2:4) on the Scalar ring (slow spin-up but needed later),
# [4:8) and [8:16) on Sync behind the rest of xt.
_W1Z_BOOT_TILES = 2


def _build_nc(caps, has_bias):
    """Build + compile the SPMD Bass program for per-slot capacities `caps`."""
    import concourse.bacc as bacc
    import concourse.bass as bass
    import concourse.mybir as mybir
    import concourse.tile as tile

    fp32 = mybir.dt.float32
    bf16 = mybir.dt.bfloat16
    f8 = mybir.dt.float8e4
    alu = mybir.AluOpType

    xoff, yoff = _slot_geometry(caps)
    XCOLS = xoff[-1]
    YROWS = yoff[-1]
    CAPMAX = max(caps)
    # DoubleRow LDWEIGHTS requires the k-pair step to be a multiple of 16
    # (s3_lw dual-fp8 AP restriction), so the fp8 ht tile pads its per-k-tile
    # column capacity.
    CAP8 = -(-CAPMAX // 16) * 16
    chunks = [_chunk_list(caps[e], e) for e in range(E_LOCAL)]

    nc = bacc.Bacc("TRN2", target_bir_lowering=False, debug=False)

    # xt/w1z/w1r/w2 are partition-major: one contiguous run per partition per
    # transfer -> 128 large DMA descriptors instead of 512-2048 small ones.
    # xt is chunk-major within a slot: [chunk0: k0|k1|k2|k3, chunk1: ...] so a
    # chunk's worth of tokens is one contiguous transfer.
    C0 = chunks[0][0]
    ZB = _W1Z_BOOT_TILES
    BOOTC = KD * C0 + ZB * KD * 128
    xt_d = nc.dram_tensor("xt", [128, KD * XCOLS], bf16, kind="ExternalInput")
    boot_d = nc.dram_tensor("boot", [128, BOOTC], bf16, kind="ExternalInput")
    w1z_d = nc.dram_tensor(
        "w1z", [128, (KF - ZB) * KD * 128], bf16, kind="ExternalInput"
    )
    w1r_d = nc.dram_tensor(
        "w1r", [E_LOCAL - 1, 128, KD * F], bf16, kind="ExternalInput"
    )
    w2_d = nc.dram_tensor("w2", [E_LOCAL, 128, KF * D], bf16, kind="ExternalInput")
    w28_d = nc.dram_tensor(
        "w28", [E_LOCAL, 128, FP8_KT * D], f8, kind="ExternalInput"
    )
    if has_bias:
        b1_d = nc.dram_tensor("b1", [E_LOCAL, 128, KF], fp32, kind="ExternalInput")
        b2_d = nc.dram_tensor("b2", [E_LOCAL, D], fp32, kind="ExternalInput")
    y_d = nc.dram_tensor("y", [YROWS, D], fp32, kind="ExternalOutput")

    with tile.TileContext(nc) as tc:
        with (
            tc.tile_pool(name="w1zpool", bufs=1) as w1zp,
            tc.tile_pool(name="wpool", bufs=2) as wp,
            tc.tile_pool(name="w2pool", bufs=4) as w2p,
            tc.tile_pool(name="rpool", bufs=1) as rp,
            tc.tile_pool(name="xpool", bufs=2) as xp,
            tc.tile_pool(name="hpool", bufs=2) as hp,
            tc.tile_pool(name="h8pool", bufs=2) as h8p,
            tc.tile_pool(name="w28pool", bufs=2) as w28p,
            tc.tile_pool(name="ypool", bufs=4) as yp,
            tc.tile_pool(name="bias", bufs=1) as bp,
            tc.tile_pool(name="psh", bufs=3, space="PSUM") as psh,
            tc.tile_pool(name="psy", bufs=3, space="PSUM") as psy,
            tc.tile_pool(name="psy8", bufs=2, space="PSUM") as psy8,
        ):
            w1_sbs = [None] * E_LOCAL
            w2_sbs = [None] * E_LOCAL
            w28_sbs = [None] * E_LOCAL
            xt_sbs = [None] * E_LOCAL

            def chunk_col0(e, ci):
                # column offset of chunk ci inside slot e's xt block
                return KD * xoff[e] + KD * sum(chunks[e][:ci])

            # --- slot-0 critical startup loads -------------------------------
            # The DMA issue order below IS the delivery order per ring (FIFO
            # start + packet round-robin), arranged by first-use time.
            cap0 = caps[0]
            assert cap0 > 0 and len(chunks[0]) == 2
            boot_sb = w1zp.tile([128, BOOTC], bf16, name="boot")
            nc.sync.dma_start(out=boot_sb[:], in_=boot_d[:])
            w1z_sb = w1zp.tile([128, KF - ZB, KD, 128], bf16, name="w1z")
            w1z_v = w1z_sb.rearrange("p a b c -> p (a b c)")

            def w1z_block(t0, t1, eng):
                # f-tiles [t0:t1) of slot-0 w1; tiles 0..ZB-1 live in boot
                eng.dma_start(
                    out=w1z_v[:, (t0 - ZB) * KD * 128:(t1 - ZB) * KD * 128],
                    in_=w1z_d[:, (t0 - ZB) * KD * 128:(t1 - ZB) * KD * 128],
                )

            w1z_block(2, 4, nc.scalar)
            # xt slot-0 chunk 1 (chunk 0 came in boot)
            cs1 = chunks[0][1]
            xt0_sb = xp.tile([128, KD * cs1], bf16, tag="xt")
            nc.sync.dma_start(
                out=xt0_sb[:],
                in_=xt_d[:, KD * xoff[0] + KD * C0:KD * xoff[1]],
            )
            w1z_block(4, 8, nc.sync)
            w1z_block(8, 16, nc.sync)
            w2z_sb = w2p.tile([128, KF * D], bf16, tag="w2")
            nc.sync.dma_start(out=w2z_sb[:], in_=w2_d[0])
            w28z_sb = w28p.tile([128, FP8_KT, D], f8, tag="w28")
            nc.sync.dma_start(
                out=w28z_sb.rearrange("p a b -> p (a b)"), in_=w28_d[0]
            )
            xt_sbs[0], w1_sbs[0], w2_sbs[0] = xt0_sb, w1z_sb, w2z_sb
            w28_sbs[0] = w28z_sb

            def load_slot(e):
                # Sync HWDGE ring: FIFO start order + packet-level round-robin.
                cap = caps[e]
                xt_sb = xp.tile([128, KD * cap], bf16, tag="xt")
                nc.sync.dma_start(
                    out=xt_sb[:],
                    in_=xt_d[:, KD * xoff[e]:KD * xoff[e + 1]],
                )
                w1_sb = wp.tile([128, KD, F], bf16, tag="w1")
                nc.sync.dma_start(
                    out=w1_sb.rearrange("p k f -> p (k f)"),
                    in_=w1r_d[e - 1],
                )
                w2_sb = w2p.tile([128, KF * D], bf16, tag="w2")
                nc.sync.dma_start(out=w2_sb[:], in_=w2_d[e])
                w28_sb = w28p.tile([128, FP8_KT, D], f8, tag="w28")
                nc.sync.dma_start(
                    out=w28_sb.rearrange("p a b -> p (a b)"), in_=w28_d[e]
                )
                xt_sbs[e], w1_sbs[e], w2_sbs[e] = xt_sb, w1_sb, w2_sb
                w28_sbs[e] = w28_sb

            if has_bias:
                # biases (small / off critical path; on the gpsimd queue)
                b1_sb = bp.tile([128, E_LOCAL, KF], fp32)
                nc.gpsimd.dma_start(
                    out=b1_sb[:], in_=b1_d[:].rearrange("e p f -> p e f")
                )
                b2_sb = bp.tile([128, E_LOCAL, D], fp32)
                b2_ap = b2_d[:]
                b2_bc = bass.AP(
                    tensor=b2_ap.tensor,
                    offset=b2_ap.offset,
                    ap=[[0, 128]] + [list(a) for a in b2_ap.ap],
                )
                nc.gpsimd.dma_start(out=b2_sb[:], in_=b2_bc)

            # Layer-2 partial tiles cost a full 16x512-cycle pass no matter
            # how few tokens they hold. Pack the remainder tokens of 3-slot
            # windows into <=32-token column groups and run up to 4 groups
            # concurrently in one PE pass (column tiling, tile_position
            # derived automatically from the PSUM base partition).
            WINDOWS = [(0, 3), (3, 6)]  # slots 6-7 keep their partial tiles
            packed = set()
            win_pieces = {}
            for w0, w1e in WINDOWS:
                pieces = []  # (slot, ht_col0, m, rbase)
                rbase = sum(
                    -(-(caps[s] % 128) // 32) * 32
                    for ww0, ww1 in WINDOWS if (ww0, ww1) < (w0, w1e)
                    for s in range(ww0, ww1) if caps[s] % 128
                )
                for s in range(w0, min(w1e, E_LOCAL)):
                    rem = caps[s] % 128
                    if rem == 0 or caps[s] == 0:
                        continue
                    full = caps[s] // 128
                    off = 0
                    while off < rem:
                        m = min(32, rem - off)
                        pieces.append((s, full * 128 + off, m, rbase + off))
                        off += m
                    rbase += -(-rem // 32) * 32
                n_passes = -(-len(pieces) // 4)
                n_slots = len({p[0] for p in pieces})
                if pieces and n_passes < n_slots:
                    win_pieces[(w0, w1e)] = pieces
                    packed.update({p[0] for p in pieces})
            RTOT = sum(
                -(-(caps[s] % 128) // 32) * 32
                for w0, w1e in win_pieces
                for s in range(w0, min(w1e, E_LOCAL)) if caps[s] % 128
            )
            r_sb = rp.tile([128, KF, max(RTOT, 32)], bf16, name="r_sb") if win_pieces else None
            ht_sbs = {}

            def epilogue(y_sb, py, rows, e, r0=0):
                if has_bias:
                    nc.vector.tensor_add(
                        y_sb[r0:r0 + rows, :],
                        py[r0:r0 + rows, :],
                        b2_sb[r0:r0 + rows, e, :],
                    )
                else:
                    nc.vector.tensor_copy(
                        y_sb[r0:r0 + rows, :], py[r0:r0 + rows, :]
                    )

            def epilogue2(y_sb, y8_sb, py, py8, rows, e, d0, d1):
                # y = py + py8/_W28_SCALE (+ b2). A DVE op may read only one
                # PSUM operand, so the scalar engine (idle during the L2
                # phase) descales py8 into SBUF first.
                nc.scalar.activation(
                    out=y8_sb[:rows, d0:d1],
                    in_=py8[:rows, d0:d1],
                    func=mybir.ActivationFunctionType.Copy,
                    scale=1.0 / _W28_SCALE,
                )
                nc.vector.tensor_add(
                    y_sb[:rows, d0:d1],
                    y8_sb[:rows, d0:d1],
                    py[:rows, d0:d1],
                )
                if has_bias:
                    nc.vector.tensor_add(
                        y_sb[:rows, d0:d1],
                        y_sb[:rows, d0:d1],
                        b2_sb[:rows, e, d0:d1],
                    )

            def packed_pass(w0, w1e):
                pieces = win_pieces[(w0, w1e)]
                for i0 in range(0, len(pieces), 4):
                    grp = pieces[i0:i0 + 4]
                    py = psy.tile([128, D], fp32, tag="py")
                    for k in range(KF):
                        for gi, (s, hc0, m, rb) in enumerate(grp):
                            nc.tensor.matmul(
                                py[32 * gi:32 * gi + m, :],
                                lhsT=r_sb[:, k, rb:rb + m],
                                rhs=w2_sbs[s][:, k * D:(k + 1) * D],
                                start=(k == 0),
                                stop=(k == KF - 1),
                                tile_position=(0, 32 * gi),
                            )
                    y_sb = yp.tile([128, D], fp32, tag="ysb")
                    for gi, (s, hc0, m, rb) in enumerate(grp):
                        epilogue(y_sb, py, m, s, r0=32 * gi)
                        row0 = yoff[s] + (caps[s] // 128) * 128 + (hc0 - (caps[s] // 128) * 128)
                        nc.sync.dma_start(
                            out=y_d[row0:row0 + m, :],
                            in_=y_sb[32 * gi:32 * gi + m, :],
                        )

            for e in range(E_LOCAL):
                cap = caps[e]
                if cap == 0:
                    continue
                if e + 1 < E_LOCAL and caps[e + 1] > 0:
                    load_slot(e + 1)
                w1_sb, w2_sb, xt_sb = w1_sbs[e], w2_sbs[e], xt_sbs[e]

                def l1_lhsT(k, f):
                    if e == 0:
                        if f < ZB:
                            b0 = KD * C0 + f * KD * 128 + k * 128
                            return boot_sb[:, b0:b0 + 128]
                        return w1_sb[:, f - ZB, k, :]
                    return w1_sb[:, k, f * 128:(f + 1) * 128]

                def l1_rhs(ci, cs, k):
                    if e == 0:
                        if ci == 0:
                            return boot_sb[:, k * cs:(k + 1) * cs]
                        return xt_sb[:, k * cs:(k + 1) * cs]
                    xb = KD * sum(chunks[e][:ci])
                    return xt_sb[:, xb + k * cs:xb + (k + 1) * cs]

                # layer 1: HT[f-tile, tok] = gelu(w1_tile.T @ XT + b1)
                ht_sb = hp.tile([128, KF, CAPMAX], bf16, tag="ht")
                ht8_sb = h8p.tile([128, FP8_KT, CAP8], f8, tag="ht8")
                for f in range(KF):
                    for ci, cs in enumerate(chunks[e]):
                        c0 = sum(chunks[e][:ci])
                        ph = psh.tile([128, 512], fp32, tag="ph")
                        for k in range(KD):
                            nc.tensor.matmul(
                                ph[:, :cs],
                                lhsT=l1_lhsT(k, f),
                                rhs=l1_rhs(ci, cs, k),
                                start=(k == 0),
                                stop=(k == KD - 1),
                            )
                        nc.scalar.activation(
                            out=ht_sb[:, f, c0:c0 + cs],
                            in_=ph[:, :cs],
                            func=mybir.ActivationFunctionType.Gelu,
                            bias=(b1_sb[:, e, f:f + 1] if has_bias else 0.0),
                            scale=1.0,
                        )
                        if f < FP8_KT:
                            # fp8 copy of layer-2 k-tiles 0..FP8_KT-1 for the
                            # DoubleRow matmul (DVE cast, off critical path)
                            nc.vector.tensor_copy(
                                ht8_sb[:, f, c0:c0 + cs],
                                ht_sb[:, f, c0:c0 + cs],
                            )

                if e in packed:
                    rem = cap % 128
                    rb0 = None
                    for (s, hc0, m, rb) in [p for w in win_pieces.values() for p in w]:
                        if s == e:
                            rb0 = rb
                            break
                    nc.vector.tensor_copy(
                        r_sb[:, :, rb0:rb0 + rem],
                        ht_sb[:, :, (cap // 128) * 128:cap],
                    )

                # layer 2: Y[t-tile, :] = HT_tile.T @ w2 + b2. k-tiles
                # 0..FP8_KT-1 run as one fp8 DoubleRow matmul into py8
                # (scaled by _W28_SCALE); the rest accumulate in py.
                w28_sb = w28_sbs[e]
                NT = cap // 128 if e in packed else -(-cap // 128)
                for t in range(NT):
                    tt = min(128, cap - t * 128)
                    py = psy.tile([128, D], fp32, tag="py")
                    py8 = psy8.tile([128, D], fp32, tag="py8")
                    last_tile = e == E_LOCAL - 1 and t == NT - 1
                    # Tail critical path: the very last tile runs in two
                    # column halves so the first half's PSUM->SBUF move and
                    # output DMA overlap the second half's matmuls, and the
                    # final DMAs are split across both HWDGE rings (each
                    # stripes packets over all 16 SDMA engines; the gpsimd
                    # software queue does not).
                    col_splits = [(0, 256), (256, 512)] if last_tile else [(0, D)]
                    for (d0, d1) in col_splits:
                        nc.tensor.matmul(
                            py8[:tt, d0:d1],
                            lhsT=ht8_sb[:, :, t * 128:t * 128 + tt],
                            rhs=w28_sb[:, :, d0:d1],
                            start=True,
                            stop=True,
                            perf_mode=mybir.MatmulPerfMode.DoubleRow,
                        )
                        for k in range(FP8_KT, KF):
                            nc.tensor.matmul(
                                py[:tt, d0:d1],
                                lhsT=ht_sb[:, k, t * 128:t * 128 + tt],
                                rhs=w2_sb[:, k * D + d0:k * D + d1],
                                start=(k == FP8_KT),
                                stop=(k == KF - 1),
                            )
                    y_sb = yp.tile([128, D], fp32, tag="ysb")
                    y8_sb = yp.tile([128, D], fp32, tag="y8sb")
                    if last_tile:
                        # The first column half's PSUM->SBUF move runs while
                        # the second half's matmuls stream; after the second
                        # move, the output leaves as full-row DMAs (2 KB
                        # contiguous DRAM rows — column-sliced 1 KB strided
                        # writes retire ~3x slower) split across both rings.
                        row0 = yoff[e] + t * 128
                        epilogue2(y_sb, y8_sb, py, py8, tt, e, 0, 256)
                        epilogue2(y_sb, y8_sb, py, py8, tt, e, 256, 512)
                        h0 = (tt + 1) // 2
                        nc.sync.dma_start(
                            out=y_d[row0:row0 + h0, :],
                            in_=y_sb[:h0, :],
                        )
                        nc.scalar.dma_start(
                            out=y_d[row0 + h0:row0 + tt, :],
                            in_=y_sb[h0:tt, :],
                        )
                    else:
                        epilogue2(y_sb, y8_sb, py, py8, tt, e, 0, D)
                        nc.sync.dma_start(
                            out=y_d[yoff[e] + t * 128: yoff[e] + t * 128 + tt, :],
                            in_=y_sb[:tt, :],
                        )

                for (w0, w1e) in list(win_pieces):
                    if e == min(w1e, E_LOCAL) - 1:
                        packed_pass(w0, w1e)

    nc.compile()
    return nc


def _get_nc(caps, has_bias):
    key = (tuple(caps), has_bias)
    if key not in _nc_cache:
        _nc_cache[key] = _build_nc(tuple(caps), has_bias)
    return _nc_cache[key]


def kernel(**inputs):
    x = np.asarray(inputs["inputs"], dtype=np.float32)
    disp = np.asarray(inputs["dispatch_order"])
    w1 = np.asarray(inputs["w1"], dtype=np.float32)
    b1 = np.asarray(inputs["b1"], dtype=np.float32)
    w2 = np.asarray(inputs["w2"], dtype=np.float32)
    b2 = np.asarray(inputs["b2"], dtype=np.float32)

    B, S, Dd = x.shape
    assert Dd == D
    T = B * S
    xf = x.reshape(T, D)
    e = disp.astype(np.int64)
    has_bias = bool(np.any(b1) or np.any(b2))

    counts = np.bincount(e, minlength=NUM_EXPERTS)
    order = np.argsort(e, kind="stable")
    xs = xf[order]  # tokens grouped by expert, original order within expert
    offs = np.zeros(NUM_EXPERTS + 1, dtype=np.int64)
    np.cumsum(counts, out=offs[1:])

    # assign experts to (slot, core): slot j of core c gets the (8j+c)-th
    # most-loaded expert -> tight per-slot caps, balanced cores
    by_load = np.argsort(-counts, kind="stable")
    slot_expert = by_load.reshape(E_LOCAL, N_CORES)  # [slot, core] -> expert id
    caps = tuple(int(counts[slot_expert[j]].max()) for j in range(E_LOCAL))
    xoff, yoff = _slot_geometry(caps)
    chunks = [_chunk_list(caps[j], j) for j in range(E_LOCAL)]

    # weights in device layout (partition-major; slot-0 w1 is additionally
    # f-tile-major so progressive f-blocks are contiguous per partition)
    w1b = w1.astype(_BF16).reshape(NUM_EXPERTS, KD, 128, F)
    w1p = np.ascontiguousarray(
        w1b.transpose(0, 2, 1, 3).reshape(NUM_EXPERTS, 128, KD * F)
    )
    w1zp = np.ascontiguousarray(
        w1b.reshape(NUM_EXPERTS, KD, 128, KF, 128)
        .transpose(0, 2, 3, 1, 4).reshape(NUM_EXPERTS, 128, KF * KD * 128)
    )
    w2p = np.ascontiguousarray(
        w2.astype(_BF16).reshape(NUM_EXPERTS, KF, 128, D)
        .transpose(0, 2, 1, 3).reshape(NUM_EXPERTS, 128, KF * D)
    )
    # fp8 copy of w2 k-tiles 0..FP8_KT-1 (F rows 0:FP8_KT*128), pre-scaled
    w28p = np.ascontiguousarray(
        (w2[:, :FP8_KT * 128, :] * _W28_SCALE).astype(_F8)
        .reshape(NUM_EXPERTS, FP8_KT, 128, D)
        .transpose(0, 2, 1, 3).reshape(NUM_EXPERTS, 128, FP8_KT * D)
    )
    b1r = np.ascontiguousarray(
        b1.reshape(NUM_EXPERTS, KF, 128).transpose(0, 2, 1)
    )  # [E, 128, KF]
    xsb = xs.astype(_BF16)

    in_maps = []
    for c in range(N_CORES):
        eids = [int(slot_expert[j, c]) for j in range(E_LOCAL)]
        xt = np.zeros((128, KD * xoff[-1]), dtype=_BF16)
        for j, ei in enumerate(eids):
            cnt = int(counts[ei])
            cap = caps[j]
            if cnt:
                xe = xsb[offs[ei]:offs[ei + 1]]  # [cnt, D]
                base = KD * xoff[j]
                for ci, cs in enumerate(chunks[j]):
                    t0 = sum(chunks[j][:ci])
                    n = max(0, min(cs, cnt - t0))
                    if n == 0:
                        continue
                    xc = xe[t0:t0 + n]  # [n, D]
                    xtj = xc.T.reshape(KD, 128, n).transpose(1, 0, 2)
                    cb = base + KD * t0
                    for k in range(KD):
                        xt[:, cb + k * cs:cb + k * cs + n] = xtj[:, k, :]
        # boot transfer: slot-0 xt chunk 0 + slot-0 w1 f-tiles [0:ZB)
        ZB = _W1Z_BOOT_TILES
        C0 = chunks[0][0]
        boot = np.concatenate(
            [xt[:, KD * xoff[0]:KD * xoff[0] + KD * C0],
             w1zp[eids[0]][:, :ZB * KD * 128]], axis=1
        )
        m = {
            "xt": xt,
            "boot": np.ascontiguousarray(boot),
            "w1z": np.ascontiguousarray(w1zp[eids[0]][:, ZB * KD * 128:]),
            "w1r": np.ascontiguousarray(w1p[eids[1:]]),
            "w2": np.ascontiguousarray(w2p[eids]),
            "w28": np.ascontiguousarray(w28p[eids]),
        }
        if has_bias:
            m["b1"] = np.ascontiguousarray(b1r[eids])
            m["b2"] = np.ascontiguousarray(b2[eids])
        in_maps.append(m)

    nc = _get_nc(caps, has_bias)
    global _last_in_maps
    _last_in_maps = in_maps
    from concourse.bass_utils import run_bass_kernel_spmd

    res = run_bass_kernel_spmd(nc, in_maps, core_ids=list(range(N_CORES)))

    out_sorted = np.empty((T, D), dtype=np.float32)
    for c in range(N_CORES):
        y = res.results[c]["y"]
        for j in range(E_LOCAL):
            ei = int(slot_expert[j, c])
            cnt = int(counts[ei])
            if cnt:
                out_sorted[offs[ei]:offs[ei + 1]] = y[yoff[j]:yoff[j] + cnt]

    out = np.empty((T, D), dtype=np.float32)
    out[order] = out_sorted
    return out.reshape(B, S, D)


# revision 34
# speedup vs baseline: 1.1094x; 1.1094x over previous
"""MoE expert-parallel kernel for Trainium2 (8 NeuronCores).

Strategy:
  - Host: route tokens to experts (stable sort by dispatch_order). Experts are
    assigned to (core, slot) pairs by descending token count: slot j of core c
    gets the (8*j + c)-th most-loaded expert, so all cores see nearly identical
    work and slot j's capacity cap_j = max over cores of its count (tight).
  - Device (SPMD, 8 cores, 8 expert slots/core):
    per slot: HT = gelu(w1^T-tiled @ XT + b1) computed transposed [F, tokens],
    then Y = HT^T @ w2 + b2 [tokens, D]; bf16 operands, fp32 PSUM accumulation.
  - Host: scatter per-expert outputs back to original token order.

Startup critical path: the first matmul needs only slot-0's xt and the first
f-tiles of slot-0's w1. Slot-0 w1 is stored f-tile-major ([128, KF, KD, 128])
so progressive f-blocks are contiguous per partition, and the early blocks go
on the Scalar HWDGE ring while xt chunks go on the Sync ring — the two DGEs
generate descriptors in parallel and neither queues behind the other.

Exit critical path: the final tile's y rows are split across the Sync and
Scalar HWDGE rings (both stripe packets over all 16 SDMA engines); the GpSimd
software queue is avoided (it lumps a whole transfer onto one engine).

No cross-core collectives: each core owns a disjoint set of experts, hence a
disjoint set of output token rows.
"""

import sys

import numpy as np
import ml_dtypes

for _p in ("/opt/trn_rl_repo",):
    if _p not in sys.path:
        sys.path.insert(0, _p)

_BF16 = ml_dtypes.bfloat16
_F8 = getattr(ml_dtypes, "float8_e4m3", ml_dtypes.float8_e4m3fn)

NUM_EXPERTS = 64
N_CORES = 8
E_LOCAL = NUM_EXPERTS // N_CORES  # 8 expert slots per core
D = 512
F = 2048
KD = D // 128   # 4 contraction tiles for layer 1
KF = F // 128   # 16 contraction tiles for layer 2

# Layer-2 k-tiles 0-1 run as one fp8 DoubleRow matmul (2x PE rate). The fp8
# quantization error on 1/8 of the contraction keeps the end-to-end max
# relative error at ~1.6e-2 (vs 3.5e-3 pure-bf16), under the 2e-2 budget.
# w2's fp8 copy is pre-scaled by _W28_SCALE (its values ~0.02 would land in
# e4m3's denormal range unscaled); the partial sum is descaled in the DVE
# epilogue, which is why it accumulates in a separate PSUM tile.
FP8_KT = 2
_W28_SCALE = 64.0

_nc_cache = {}


def _chunk_list(cap, e):
    """Layer-1 token chunks per slot (PSUM free dim <= 512 fp32).

    Balanced halves for cap > 512: a tiny trailing chunk would pay a full
    weight-load pass for a handful of columns.
    """
    if cap == 0:
        return []
    if cap <= 512:
        return [cap]
    h = (cap + 1) // 2
    return [h, cap - h]


def _slot_geometry(caps):
    """Per-slot column offsets for xt and row offsets for y."""
    xoff = [0]
    yoff = [0]
    for c in caps:
        xoff.append(xoff[-1] + c)
        yoff.append(yoff[-1] + (-(-c // 128)) * 128)
    return xoff, yoff


# Slot-0 startup: the first xt chunk and w1 f-tiles 0-1 are fused into one
# "boot" transfer (~4.3 KB per-partition descriptors — big descriptors are
# what the SDMA engines sustain high rates on) issued first on the Sync
# ring, so a single early completion unblocks the first matmuls. Later
# f-blocks: [2:4) on the Scalar ring (slow spin-up but needed later),
# [4:8) and [8:16) on Sync behind the rest of xt.
_W1Z_BOOT_TILES = 2


def _build_nc(caps, has_bias):
    """Build + compile the SPMD Bass program for per-slot capacities `caps`."""
    import concourse.bacc as bacc
    import concourse.bass as bass
    import concourse.mybir as mybir
    import concourse.tile as tile

    fp32 = mybir.dt.float32
    bf16 = mybir.dt.bfloat16
    f8 = mybir.dt.float8e4
    alu = mybir.AluOpType

    xoff, yoff = _slot_geometry(caps)
    XCOLS = xoff[-1]
    YROWS = yoff[-1]
    CAPMAX = max(caps)
    # DoubleRow LDWEIGHTS requires the k-pair step to be a multiple of 16
    # (s3_lw dual-fp8 AP restriction), so the fp8 ht tile pads its per-k-tile
    # column capacity.
    CAP8 = -(-CAPMAX // 16) * 16
    chunks = [_chunk_list(caps[e], e) for e in range(E_LOCAL)]

    nc = bacc.Bacc("TRN2", target_bir_lowering=False, debug=False)

    # xt/w1z/w1r/w2 are partition-major: one contiguous run per partition per
    # transfer -> 128 large DMA descriptors instead of 512-2048 small ones.
    # xt is chunk-major within a slot: [chunk0: k0|k1|k2|k3, chunk1: ...] so a
    # chunk's worth of tokens is one contiguous transfer.
    C0 = chunks[0][0]
    ZB = _W1Z_BOOT_TILES
    BOOTC = KD * C0 + ZB * KD * 128
    xt_d = nc.dram_tensor("xt", [128, KD * XCOLS], bf16, kind="ExternalInput")
    boot_d = nc.dram_tensor("boot", [128, BOOTC], bf16, kind="ExternalInput")
    w1z_d = nc.dram_tensor(
        "w1z", [128, (KF - ZB) * KD * 128], bf16, kind="ExternalInput"
    )
    w1r_d = nc.dram_tensor(
        "w1r", [E_LOCAL - 1, 128, KD * F], bf16, kind="ExternalInput"
    )
    w2_d = nc.dram_tensor("w2", [E_LOCAL, 128, KF * D], bf16, kind="ExternalInput")
    w28_d = nc.dram_tensor(
        "w28", [E_LOCAL, 128, FP8_KT * D], f8, kind="ExternalInput"
    )
    if has_bias:
        b1_d = nc.dram_tensor("b1", [E_LOCAL, 128, KF], fp32, kind="ExternalInput")
        b2_d = nc.dram_tensor("b2", [E_LOCAL, D], fp32, kind="ExternalInput")
    y_d = nc.dram_tensor("y", [YROWS, D], fp32, kind="ExternalOutput")

    with tile.TileContext(nc) as tc:
        with (
            tc.tile_pool(name="w1zpool", bufs=1) as w1zp,
            tc.tile_pool(name="wpool", bufs=2) as wp,
            tc.tile_pool(name="w2pool", bufs=4) as w2p,
            tc.tile_pool(name="rpool", bufs=1) as rp,
            tc.tile_pool(name="xpool", bufs=2) as xp,
            tc.tile_pool(name="hpool", bufs=2) as hp,
            tc.tile_pool(name="h8pool", bufs=2) as h8p,
            tc.tile_pool(name="w28pool", bufs=2) as w28p,
            tc.tile_pool(name="ypool", bufs=4) as yp,
            tc.tile_pool(name="bias", bufs=1) as bp,
            tc.tile_pool(name="psh", bufs=3, space="PSUM") as psh,
            tc.tile_pool(name="psy", bufs=3, space="PSUM") as psy,
            tc.tile_pool(name="psy8", bufs=2, space="PSUM") as psy8,
        ):
            w1_sbs = [None] * E_LOCAL
            w2_sbs = [None] * E_LOCAL
            w28_sbs = [None] * E_LOCAL
            xt_sbs = [None] * E_LOCAL

            def chunk_col0(e, ci):
                # column offset of chunk ci inside slot e's xt block
                return KD * xoff[e] + KD * sum(chunks[e][:ci])

            # --- slot-0 critical startup loads -------------------------------
            # The DMA issue order below IS the delivery order per ring (FIFO
            # start + packet round-robin), arranged by first-use time.
            cap0 = caps[0]
            assert cap0 > 0 and len(chunks[0]) == 2
            boot_sb = w1zp.tile([128, BOOTC], bf16, name="boot")
            nc.sync.dma_start(out=boot_sb[:], in_=boot_d[:])
            w1z_sb = w1zp.tile([128, KF - ZB, KD, 128], bf16, name="w1z")
            w1z_v = w1z_sb.rearrange("p a b c -> p (a b c)")

            def w1z_block(t0, t1, eng):
                # f-tiles [t0:t1) of slot-0 w1; tiles 0..ZB-1 live in boot
                eng.dma_start(
                    out=w1z_v[:, (t0 - ZB) * KD * 128:(t1 - ZB) * KD * 128],
                    in_=w1z_d[:, (t0 - ZB) * KD * 128:(t1 - ZB) * KD * 128],
                )

            w1z_block(2, 4, nc.scalar)
            # xt slot-0 chunk 1 (chunk 0 came in boot)
            cs1 = chunks[0][1]
            xt0_sb = xp.tile([128, KD * cs1], bf16, tag="xt")
            nc.sync.dma_start(
                out=xt0_sb[:],
                in_=xt_d[:, KD * xoff[0] + KD * C0:KD * xoff[1]],
            )
            w1z_block(4, 8, nc.sync)
            w1z_block(8, 16, nc.sync)
            w2z_sb = w2p.tile([128, KF * D], bf16, tag="w2")
            nc.sync.dma_start(out=w2z_sb[:], in_=w2_d[0])
            w28z_sb = w28p.tile([128, FP8_KT, D], f8, tag="w28")
            nc.sync.dma_start(
                out=w28z_sb.rearrange("p a b -> p (a b)"), in_=w28_d[0]
            )
            xt_sbs[0], w1_sbs[0], w2_sbs[0] = xt0_sb, w1z_sb, w2z_sb
            w28_sbs[0] = w28z_sb

            def load_slot(e):
                # Sync HWDGE ring: FIFO start order + packet-level round-robin.
                cap = caps[e]
                xt_sb = xp.tile([128, KD * cap], bf16, tag="xt")
                nc.sync.dma_start(
                    out=xt_sb[:],
                    in_=xt_d[:, KD * xoff[e]:KD * xoff[e + 1]],
                )
                w1_sb = wp.tile([128, KD, F], bf16, tag="w1")
                nc.sync.dma_start(
                    out=w1_sb.rearrange("p k f -> p (k f)"),
                    in_=w1r_d[e - 1],
                )
                w2_sb = w2p.tile([128, KF * D], bf16, tag="w2")
                nc.sync.dma_start(out=w2_sb[:], in_=w2_d[e])
                w28_sb = w28p.tile([128, FP8_KT, D], f8, tag="w28")
                nc.sync.dma_start(
                    out=w28_sb.rearrange("p a b -> p (a b)"), in_=w28_d[e]
                )
                xt_sbs[e], w1_sbs[e], w2_sbs[e] = xt_sb, w1_sb, w2_sb
                w28_sbs[e] = w28_sb

            if has_bias:
                # biases (small / off critical path; on the gpsimd queue)
                b1_sb = bp.tile([128, E_LOCAL, KF], fp32)
                nc.gpsimd.dma_start(
                    out=b1_sb[:], in_=b1_d[:].rearrange("e p f -> p e f")
                )
                b2_sb = bp.tile([128, E_LOCAL, D], fp32)
                b2_ap = b2_d[:]
                b2_bc = bass.AP(
                    tensor=b2_ap.tensor,
                    offset=b2_ap.offset,
                    ap=[[0, 128]] + [list(a) for a in b2_ap.ap],
                )
                nc.gpsimd.dma_start(out=b2_sb[:], in_=b2_bc)

            # Layer-2 partial tiles cost a full 16x512-cycle pass no matter
            # how few tokens they hold. Pack the remainder tokens of 3-slot
            # windows into <=32-token column groups and run up to 4 groups
            # concurrently in one PE pass (column tiling, tile_position
            # derived automatically from the PSUM base partition).
            WINDOWS = [(0, 3), (3, 6)]  # slots 6-7 keep their partial tiles
            packed = set()
            win_pieces = {}
            for w0, w1e in WINDOWS:
                pieces = []  # (slot, ht_col0, m, rbase)
                rbase = sum(
                    -(-(caps[s] % 128) // 32) * 32
                    for ww0, ww1 in WINDOWS if (ww0, ww1) < (w0, w1e)
                    for s in range(ww0, ww1) if caps[s] % 128
                )
                for s in range(w0, min(w1e, E_LOCAL)):
                    rem = caps[s] % 128
                    if rem == 0 or caps[s] == 0:
                        continue
                    full = caps[s] // 128
                    off = 0
                    while off < rem:
                        m = min(32, rem - off)
                        pieces.append((s, full * 128 + off, m, rbase + off))
                        off += m
                    rbase += -(-rem // 32) * 32
                n_passes = -(-len(pieces) // 4)
                n_slots = len({p[0] for p in pieces})
                if pieces and n_passes < n_slots:
                    win_pieces[(w0, w1e)] = pieces
                    packed.update({p[0] for p in pieces})
            RTOT = sum(
                -(-(caps[s] % 128) // 32) * 32
                for w0, w1e in win_pieces
                for s in range(w0, min(w1e, E_LOCAL)) if caps[s] % 128
            )
            r_sb = rp.tile([128, KF, max(RTOT, 32)], bf16, name="r_sb") if win_pieces else None
            ht_sbs = {}

            def epilogue(y_sb, py, rows, e, r0=0):
                if has_bias:
                    nc.vector.tensor_add(
                        y_sb[r0:r0 + rows, :],
                        py[r0:r0 + rows, :],
                        b2_sb[r0:r0 + rows, e, :],
                    )
                else:
                    nc.vector.tensor_copy(
                        y_sb[r0:r0 + rows, :], py[r0:r0 + rows, :]
                    )

            def epilogue2(y_sb, y8_sb, py, py8, rows, e, d0, d1):
                # y = py + py8/_W28_SCALE (+ b2). A DVE op may read only one
                # PSUM operand, so the scalar engine (idle during the L2
                # phase) descales py8 into SBUF first.
                nc.scalar.activation(
                    out=y8_sb[:rows, d0:d1],
                    in_=py8[:rows, d0:d1],
                    func=mybir.ActivationFunctionType.Copy,
                    scale=1.0 / _W28_SCALE,
                )
                nc.vector.tensor_add(
                    y_sb[:rows, d0:d1],
                    y8_sb[:rows, d0:d1],
                    py[:rows, d0:d1],
                )
                if has_bias:
                    nc.vector.tensor_add(
                        y_sb[:rows, d0:d1],
                        y_sb[:rows, d0:d1],
                        b2_sb[:rows, e, d0:d1],
                    )

            def packed_pass(w0, w1e):
                pieces = win_pieces[(w0, w1e)]
                for i0 in range(0, len(pieces), 4):
                    grp = pieces[i0:i0 + 4]
                    py = psy.tile([128, D], fp32, tag="py")
                    for k in range(KF):
                        for gi, (s, hc0, m, rb) in enumerate(grp):
                            nc.tensor.matmul(
                                py[32 * gi:32 * gi + m, :],
                                lhsT=r_sb[:, k, rb:rb + m],
                                rhs=w2_sbs[s][:, k * D:(k + 1) * D],
                                start=(k == 0),
                                stop=(k == KF - 1),
                                tile_position=(0, 32 * gi),
                            )
                    y_sb = yp.tile([128, D], fp32, tag="ysb")
                    for gi, (s, hc0, m, rb) in enumerate(grp):
                        epilogue(y_sb, py, m, s, r0=32 * gi)
                        row0 = yoff[s] + (caps[s] // 128) * 128 + (hc0 - (caps[s] // 128) * 128)
                        nc.sync.dma_start(
                            out=y_d[row0:row0 + m, :],
                            in_=y_sb[32 * gi:32 * gi + m, :],
                        )

            for e in range(E_LOCAL):
                cap = caps[e]
                if cap == 0:
                    continue
                if e + 1 < E_LOCAL and caps[e + 1] > 0:
                    load_slot(e + 1)
                w1_sb, w2_sb, xt_sb = w1_sbs[e], w2_sbs[e], xt_sbs[e]

                def l1_lhsT(k, f):
                    if e == 0:
                        if f < ZB:
                            b0 = KD * C0 + f * KD * 128 + k * 128
                            return boot_sb[:, b0:b0 + 128]
                        return w1_sb[:, f - ZB, k, :]
                    return w1_sb[:, k, f * 128:(f + 1) * 128]

                def l1_rhs(ci, cs, k):
                    if e == 0:
                        if ci == 0:
                            return boot_sb[:, k * cs:(k + 1) * cs]
                        return xt_sb[:, k * cs:(k + 1) * cs]
                    xb = KD * sum(chunks[e][:ci])
                    return xt_sb[:, xb + k * cs:xb + (k + 1) * cs]

                # layer 1: HT[f-tile, tok] = gelu(w1_tile.T @ XT + b1)
                ht_sb = hp.tile([128, KF, CAPMAX], bf16, tag="ht")
                ht8_sb = h8p.tile([128, FP8_KT, CAP8], f8, tag="ht8")
                for f in range(KF):
                    for ci, cs in enumerate(chunks[e]):
                        c0 = sum(chunks[e][:ci])
                        ph = psh.tile([128, 512], fp32, tag="ph")
                        for k in range(KD):
                            nc.tensor.matmul(
                                ph[:, :cs],
                                lhsT=l1_lhsT(k, f),
                                rhs=l1_rhs(ci, cs, k),
                                start=(k == 0),
                                stop=(k == KD - 1),
                            )
                        nc.scalar.activation(
                            out=ht_sb[:, f, c0:c0 + cs],
                            in_=ph[:, :cs],
                            func=mybir.ActivationFunctionType.Gelu,
                            bias=(b1_sb[:, e, f:f + 1] if has_bias else 0.0),
                            scale=1.0,
                        )
                        if f < FP8_KT:
                            # fp8 copy of layer-2 k-tiles 0..FP8_KT-1 for the
                            # DoubleRow matmul (DVE cast, off critical path)
                            nc.vector.tensor_copy(
                                ht8_sb[:, f, c0:c0 + cs],
                                ht_sb[:, f, c0:c0 + cs],
                            )

                if e in packed:
                    rem = cap % 128
                    rb0 = None
                    for (s, hc0, m, rb) in [p for w in win_pieces.values() for p in w]:
                        if s == e:
                            rb0 = rb
                            break
                    nc.vector.tensor_copy(
                        r_sb[:, :, rb0:rb0 + rem],
                        ht_sb[:, :, (cap // 128) * 128:cap],
                    )

                # layer 2: Y[t-tile, :] = HT_tile.T @ w2 + b2. k-tiles
                # 0..FP8_KT-1 run as one fp8 DoubleRow matmul into py8
                # (scaled by _W28_SCALE); the rest accumulate in py.
                w28_sb = w28_sbs[e]
                NT = cap // 128 if e in packed else -(-cap // 128)
                # Tiles are processed in pairs: both fp8 DoubleRow matmuls
                # back-to-back, then both bf16 chains — halving the number of
                # PE weight-path perf-mode switches, which cost a pipeline
                # hiccup each.
                tiles = list(range(NT))
                groups = []
                t = 0
                while t < NT:
                    if t + 1 < NT and not (e == E_LOCAL - 1 and t + 1 == NT - 1):
                        groups.append([t, t + 1]); t += 2
                    else:
                        groups.append([t]); t += 1
                for grp in groups:
                  pys = {}
                  py8s = {}
                  for t in grp:
                    tt = min(128, cap - t * 128)
                    py8_t = psy8.tile([128, D], fp32, tag="py8")
                    py8s[t] = py8_t
                    nc.tensor.matmul(
                        py8s[t][:tt, :],
                        lhsT=ht8_sb[:, :, t * 128:t * 128 + tt],
                        rhs=w28_sb[:, :, :],
                        start=True,
                        stop=True,
                        perf_mode=mybir.MatmulPerfMode.DoubleRow,
                    )
                  for t in grp:
                    tt = min(128, cap - t * 128)
                    py = psy.tile([128, D], fp32, tag="py")
                    pys[t] = py
                    py8 = py8s[t]
                    last_tile = e == E_LOCAL - 1 and t == NT - 1
                    # Tail critical path: the very last tile runs in two
                    # column halves so the first half's PSUM->SBUF move and
                    # output DMA overlap the second half's matmuls, and the
                    # final DMAs are split across both HWDGE rings (each
                    # stripes packets over all 16 SDMA engines; the gpsimd
                    # software queue does not).
                    col_splits = [(0, 256), (256, 512)] if last_tile else [(0, D)]
                    for (d0, d1) in col_splits:
                        for k in range(FP8_KT, KF):
                            nc.tensor.matmul(
                                py[:tt, d0:d1],
                                lhsT=ht_sb[:, k, t * 128:t * 128 + tt],
                                rhs=w2_sb[:, k * D + d0:k * D + d1],
                                start=(k == FP8_KT),
                                stop=(k == KF - 1),
                            )
                    y_sb = yp.tile([128, D], fp32, tag="ysb")
                    y8_sb = yp.tile([128, D], fp32, tag="y8sb")
                    if last_tile:
                        # The first column half's PSUM->SBUF move runs while
                        # the second half's matmuls stream; after the second
                        # move, the output leaves as full-row DMAs (2 KB
                        # contiguous DRAM rows — column-sliced 1 KB strided
                        # writes retire ~3x slower) split across both rings.
                        row0 = yoff[e] + t * 128
                        epilogue2(y_sb, y8_sb, py, py8, tt, e, 0, 256)
                        epilogue2(y_sb, y8_sb, py, py8, tt, e, 256, 512)
                        h0 = (tt + 1) // 2
                        nc.sync.dma_start(
                            out=y_d[row0:row0 + h0, :],
                            in_=y_sb[:h0, :],
                        )
                        nc.scalar.dma_start(
                            out=y_d[row0 + h0:row0 + tt, :],
                            in_=y_sb[h0:tt, :],
                        )
                    else:
                        epilogue2(y_sb, y8_sb, py, py8, tt, e, 0, D)
                        nc.sync.dma_start(
                            out=y_d[yoff[e] + t * 128: yoff[e] + t * 128 + tt, :],
                            in_=y_sb[:tt, :],
                        )

                for (w0, w1e) in list(win_pieces):
                    if e == min(w1e, E_LOCAL) - 1:
                        packed_pass(w0, w1e)

    nc.compile()
    return nc


def _get_nc(caps, has_bias):
    key = (tuple(caps), has_bias)
    if key not in _nc_cache:
        _nc_cache[key] = _build_nc(tuple(caps), has_bias)
    return _nc_cache[key]


def kernel(**inputs):
    x = np.asarray(inputs["inputs"], dtype=np.float32)
    disp = np.asarray(inputs["dispatch_order"])
    w1 = np.asarray(inputs["w1"], dtype=np.float32)
    b1 = np.asarray(inputs["b1"], dtype=np.float32)
    w2 = np.asarray(inputs["w2"], dtype=np.float32)
    b2 = np.asarray(inputs["b2"], dtype=np.float32)

    B, S, Dd = x.shape
    assert Dd == D
    T = B * S
    xf = x.reshape(T, D)
    e = disp.astype(np.int64)
    has_bias = bool(np.any(b1) or np.any(b2))

    counts = np.bincount(e, minlength=NUM_EXPERTS)
    order = np.argsort(e, kind="stable")
    xs = xf[order]  # tokens grouped by expert, original order within expert
    offs = np.zeros(NUM_EXPERTS + 1, dtype=np.int64)
    np.cumsum(counts, out=offs[1:])

    # assign experts to (slot, core): slot j of core c gets the (8j+c)-th
    # most-loaded expert -> tight per-slot caps, balanced cores
    by_load = np.argsort(-counts, kind="stable")
    slot_expert = by_load.reshape(E_LOCAL, N_CORES)  # [slot, core] -> expert id
    caps = tuple(int(counts[slot_expert[j]].max()) for j in range(E_LOCAL))
    xoff, yoff = _slot_geometry(caps)
    chunks = [_chunk_list(caps[j], j) for j in range(E_LOCAL)]

    # weights in device layout (partition-major; slot-0 w1 is additionally
    # f-tile-major so progressive f-blocks are contiguous per partition)
    w1b = w1.astype(_BF16).reshape(NUM_EXPERTS, KD, 128, F)
    w1p = np.ascontiguousarray(
        w1b.transpose(0, 2, 1, 3).reshape(NUM_EXPERTS, 128, KD * F)
    )
    w1zp = np.ascontiguousarray(
        w1b.reshape(NUM_EXPERTS, KD, 128, KF, 128)
        .transpose(0, 2, 3, 1, 4).reshape(NUM_EXPERTS, 128, KF * KD * 128)
    )
    w2p = np.ascontiguousarray(
        w2.astype(_BF16).reshape(NUM_EXPERTS, KF, 128, D)
        .transpose(0, 2, 1, 3).reshape(NUM_EXPERTS, 128, KF * D)
    )
    # fp8 copy of w2 k-tiles 0..FP8_KT-1 (F rows 0:FP8_KT*128), pre-scaled
    w28p = np.ascontiguousarray(
        (w2[:, :FP8_KT * 128, :] * _W28_SCALE).astype(_F8)
        .reshape(NUM_EXPERTS, FP8_KT, 128, D)
        .transpose(0, 2, 1, 3).reshape(NUM_EXPERTS, 128, FP8_KT * D)
    )
    b1r = np.ascontiguousarray(
        b1.reshape(NUM_EXPERTS, KF, 128).transpose(0, 2, 1)
    )  # [E, 128, KF]
    xsb = xs.astype(_BF16)

    in_maps = []
    for c in range(N_CORES):
        eids = [int(slot_expert[j, c]) for j in range(E_LOCAL)]
        xt = np.zeros((128, KD * xoff[-1]), dtype=_BF16)
        for j, ei in enumerate(eids):
            cnt = int(counts[ei])
            cap = caps[j]
            if cnt:
                xe = xsb[offs[ei]:offs[ei + 1]]  # [cnt, D]
                base = KD * xoff[j]
                for ci, cs in enumerate(chunks[j]):
                    t0 = sum(chunks[j][:ci])
                    n = max(0, min(cs, cnt - t0))
                    if n == 0:
                        continue
                    xc = xe[t0:t0 + n]  # [n, D]
                    xtj = xc.T.reshape(KD, 128, n).transpose(1, 0, 2)
                    cb = base + KD * t0
                    for k in range(KD):
                        xt[:, cb + k * cs:cb + k * cs + n] = xtj[:, k, :]
        # boot transfer: slot-0 xt chunk 0 + slot-0 w1 f-tiles [0:ZB)
        ZB = _W1Z_BOOT_TILES
        C0 = chunks[0][0]
        boot = np.concatenate(
            [xt[:, KD * xoff[0]:KD * xoff[0] + KD * C0],
             w1zp[eids[0]][:, :ZB * KD * 128]], axis=1
        )
        m = {
            "xt": xt,
            "boot": np.ascontiguousarray(boot),
            "w1z": np.ascontiguousarray(w1zp[eids[0]][:, ZB * KD * 128:]),
            "w1r": np.ascontiguousarray(w1p[eids[1:]]),
            "w2": np.ascontiguousarray(w2p[eids]),
            "w28": np.ascontiguousarray(w28p[eids]),
        }
        if has_bias:
            m["b1"] = np.ascontiguousarray(b1r[eids])
            m["b2"] = np.ascontiguousarray(b2[eids])
        in_maps.append(m)

    nc = _get_nc(caps, has_bias)
    global _last_in_maps
    _last_in_maps = in_maps
    from concourse.bass_utils import run_bass_kernel_spmd

    res = run_bass_kernel_spmd(nc, in_maps, core_ids=list(range(N_CORES)))

    out_sorted = np.empty((T, D), dtype=np.float32)
    for c in range(N_CORES):
        y = res.results[c]["y"]
        for j in range(E_LOCAL):
            ei = int(slot_expert[j, c])
            cnt = int(counts[ei])
            if cnt:
                out_sorted[offs[ei]:offs[ei + 1]] = y[yoff[j]:yoff[j] + cnt]

    out = np.empty((T, D), dtype=np.float32)
    out[order] = out_sorted
    return out.reshape(B, S, D)


# revision 36
# speedup vs baseline: 1.1101x; 1.0007x over previous
"""MoE expert-parallel kernel for Trainium2 (8 NeuronCores).

Strategy:
  - Host: route tokens to experts (stable sort by dispatch_order). Experts are
    assigned to (core, slot) pairs by descending token count: slot j of core c
    gets the (8*j + c)-th most-loaded expert, so all cores see nearly identical
    work and slot j's capacity cap_j = max over cores of its count (tight).
  - Device (SPMD, 8 cores, 8 expert slots/core):
    per slot: HT = gelu(w1^T-tiled @ XT + b1) computed transposed [F, tokens],
    then Y = HT^T @ w2 + b2 [tokens, D]; bf16 operands, fp32 PSUM accumulation,
    with layer-2 k-tiles 0-1 as an fp8 DoubleRow matmul (see FP8_KT below).
  - Host: scatter per-expert outputs back to original token order.

Startup critical path: the first matmul needs only slot-0's xt and the first
f-tiles of slot-0's w1. Slot-0 w1 is stored f-tile-major ([128, KF, KD, 128])
so progressive f-blocks are contiguous per partition, and the early blocks go
on the Scalar HWDGE ring while xt chunks go on the Sync ring — the two DGEs
generate descriptors in parallel and neither queues behind the other.

Exit critical path: the final tile's y rows are split across the Sync and
Scalar HWDGE rings (both stripe packets over all 16 SDMA engines); the GpSimd
software queue is avoided (it lumps a whole transfer onto one engine).

No cross-core collectives: each core owns a disjoint set of experts, hence a
disjoint set of output token rows.
"""

import sys

import numpy as np
import ml_dtypes

for _p in ("/opt/trn_rl_repo",):
    if _p not in sys.path:
        sys.path.insert(0, _p)

_BF16 = ml_dtypes.bfloat16
_F8 = getattr(ml_dtypes, "float8_e4m3", ml_dtypes.float8_e4m3fn)

NUM_EXPERTS = 64
N_CORES = 8
E_LOCAL = NUM_EXPERTS // N_CORES  # 8 expert slots per core
D = 512
F = 2048
KD = D // 128   # 4 contraction tiles for layer 1
KF = F // 128   # 16 contraction tiles for layer 2

# Layer-2 k-tiles 0-1 run as one fp8 DoubleRow matmul (2x PE rate). The fp8
# quantization error on 1/8 of the contraction keeps the end-to-end max
# relative error at ~1.6e-2 (vs 3.5e-3 pure-bf16), under the 2e-2 budget.
# w2's fp8 copy is pre-scaled by _W28_SCALE (its values ~0.02 would land in
# e4m3's denormal range unscaled); the partial sum accumulates in a separate
# PSUM tile and is descaled by the scalar engine in the epilogue.
FP8_KT = 2
_W28_SCALE = 64.0

_nc_cache = {}


def _chunk_list(cap, e):
    """Layer-1 token chunks per slot (PSUM free dim <= 512 fp32).

    Balanced halves for cap > 512: a tiny trailing chunk would pay a full
    weight-load pass for a handful of columns.
    """
    if cap == 0:
        return []
    if cap <= 512:
        return [cap]
    h = (cap + 1) // 2
    return [h, cap - h]


def _slot_geometry(caps):
    """Per-slot column offsets for xt and row offsets for y."""
    xoff = [0]
    yoff = [0]
    for c in caps:
        xoff.append(xoff[-1] + c)
        yoff.append(yoff[-1] + (-(-c // 128)) * 128)
    return xoff, yoff


# Slot-0 startup: the first xt chunk and w1 f-tiles 0-1 are fused into one
# "boot" transfer (~4.3 KB per-partition descriptors — big descriptors are
# what the SDMA engines sustain high rates on) issued first on the Sync
# ring, so a single early completion unblocks the first matmuls. Later
# f-blocks: [2:4) on the Scalar ring (slow spin-up but needed later),
# [4:8) and [8:16) on Sync behind the rest of xt.
_W1Z_BOOT_TILES = 2


def _build_nc(caps, has_bias):
    """Build + compile the SPMD Bass program for per-slot capacities `caps`."""
    import concourse.bacc as bacc
    import concourse.bass as bass
    import concourse.mybir as mybir
    import concourse.tile as tile

    fp32 = mybir.dt.float32
    bf16 = mybir.dt.bfloat16
    f8 = mybir.dt.float8e4
    alu = mybir.AluOpType

    xoff, yoff = _slot_geometry(caps)
    XCOLS = xoff[-1]
    YROWS = yoff[-1]
    CAPMAX = max(caps)
    # DoubleRow LDWEIGHTS requires the k-pair step to be a multiple of 16
    # (s3_lw dual-fp8 AP restriction), so the fp8 ht tile pads its per-k-tile
    # column capacity.
    CAP8 = -(-CAPMAX // 16) * 16
    chunks = [_chunk_list(caps[e], e) for e in range(E_LOCAL)]

    nc = bacc.Bacc("TRN2", target_bir_lowering=False, debug=False)

    # xt/w1z/w1r/w2 are partition-major: one contiguous run per partition per
    # transfer -> 128 large DMA descriptors instead of 512-2048 small ones.
    # xt is chunk-major within a slot: [chunk0: k0|k1|k2|k3, chunk1: ...] so a
    # chunk's worth of tokens is one contiguous transfer.
    C0 = chunks[0][0]
    ZB = _W1Z_BOOT_TILES
    BOOTC = KD * C0 + ZB * KD * 128
    xt_d = nc.dram_tensor("xt", [128, KD * XCOLS], bf16, kind="ExternalInput")
    boot_d = nc.dram_tensor("boot", [128, BOOTC], bf16, kind="ExternalInput")
    w1z_d = nc.dram_tensor(
        "w1z", [128, (KF - ZB) * KD * 128], bf16, kind="ExternalInput"
    )
    w1r_d = nc.dram_tensor(
        "w1r", [E_LOCAL - 1, 128, KD * F], bf16, kind="ExternalInput"
    )
    w2_d = nc.dram_tensor("w2", [E_LOCAL, 128, KF * D], bf16, kind="ExternalInput")
    w28_d = nc.dram_tensor(
        "w28", [E_LOCAL, 128, FP8_KT * D], f8, kind="ExternalInput"
    )
    if has_bias:
        b1_d = nc.dram_tensor("b1", [E_LOCAL, 128, KF], fp32, kind="ExternalInput")
        b2_d = nc.dram_tensor("b2", [E_LOCAL, D], fp32, kind="ExternalInput")
    y_d = nc.dram_tensor("y", [YROWS, D], fp32, kind="ExternalOutput")

    with tile.TileContext(nc) as tc:
        with (
            tc.tile_pool(name="w1zpool", bufs=1) as w1zp,
            tc.tile_pool(name="wpool", bufs=2) as wp,
            tc.tile_pool(name="w2pool", bufs=4) as w2p,
            tc.tile_pool(name="rpool", bufs=1) as rp,
            tc.tile_pool(name="xpool", bufs=2) as xp,
            tc.tile_pool(name="hpool", bufs=2) as hp,
            tc.tile_pool(name="h8pool", bufs=2) as h8p,
            tc.tile_pool(name="w28pool", bufs=2) as w28p,
            tc.tile_pool(name="ypool", bufs=4) as yp,
            tc.tile_pool(name="bias", bufs=1) as bp,
            tc.tile_pool(name="psh", bufs=3, space="PSUM") as psh,
            tc.tile_pool(name="psy", bufs=3, space="PSUM") as psy,
            tc.tile_pool(name="psy8", bufs=2, space="PSUM") as psy8,
        ):
            w1_sbs = [None] * E_LOCAL
            w2_sbs = [None] * E_LOCAL
            w28_sbs = [None] * E_LOCAL
            xt_sbs = [None] * E_LOCAL

            def chunk_col0(e, ci):
                # column offset of chunk ci inside slot e's xt block
                return KD * xoff[e] + KD * sum(chunks[e][:ci])

            # --- slot-0 critical startup loads -------------------------------
            # The DMA issue order below IS the delivery order per ring (FIFO
            # start + packet round-robin), arranged by first-use time.
            cap0 = caps[0]
            assert cap0 > 0 and len(chunks[0]) == 2
            boot_sb = w1zp.tile([128, BOOTC], bf16, name="boot")
            nc.sync.dma_start(out=boot_sb[:], in_=boot_d[:])
            w1z_sb = w1zp.tile([128, KF - ZB, KD, 128], bf16, name="w1z")
            w1z_v = w1z_sb.rearrange("p a b c -> p (a b c)")

            def w1z_block(t0, t1, eng):
                # f-tiles [t0:t1) of slot-0 w1; tiles 0..ZB-1 live in boot
                eng.dma_start(
                    out=w1z_v[:, (t0 - ZB) * KD * 128:(t1 - ZB) * KD * 128],
                    in_=w1z_d[:, (t0 - ZB) * KD * 128:(t1 - ZB) * KD * 128],
                )

            w1z_block(2, 4, nc.scalar)
            # xt slot-0 chunk 1 (chunk 0 came in boot)
            cs1 = chunks[0][1]
            xt0_sb = xp.tile([128, KD * cs1], bf16, tag="xt")
            nc.sync.dma_start(
                out=xt0_sb[:],
                in_=xt_d[:, KD * xoff[0] + KD * C0:KD * xoff[1]],
            )
            w1z_block(4, 8, nc.sync)
            w1z_block(8, 16, nc.sync)
            w2z_sb = w2p.tile([128, KF * D], bf16, tag="w2")
            nc.sync.dma_start(out=w2z_sb[:], in_=w2_d[0])
            w28z_sb = w28p.tile([128, FP8_KT, D], f8, tag="w28")
            nc.sync.dma_start(
                out=w28z_sb.rearrange("p a b -> p (a b)"), in_=w28_d[0]
            )
            xt_sbs[0], w1_sbs[0], w2_sbs[0] = xt0_sb, w1z_sb, w2z_sb
            w28_sbs[0] = w28z_sb

            def load_slot(e):
                # Sync HWDGE ring: FIFO start order + packet-level round-robin.
                cap = caps[e]
                xt_sb = xp.tile([128, KD * cap], bf16, tag="xt")
                nc.sync.dma_start(
                    out=xt_sb[:],
                    in_=xt_d[:, KD * xoff[e]:KD * xoff[e + 1]],
                )
                w1_sb = wp.tile([128, KD, F], bf16, tag="w1")
                nc.sync.dma_start(
                    out=w1_sb.rearrange("p k f -> p (k f)"),
                    in_=w1r_d[e - 1],
                )
                w2_sb = w2p.tile([128, KF * D], bf16, tag="w2")
                nc.sync.dma_start(out=w2_sb[:], in_=w2_d[e])
                w28_sb = w28p.tile([128, FP8_KT, D], f8, tag="w28")
                nc.sync.dma_start(
                    out=w28_sb.rearrange("p a b -> p (a b)"), in_=w28_d[e]
                )
                xt_sbs[e], w1_sbs[e], w2_sbs[e] = xt_sb, w1_sb, w2_sb
                w28_sbs[e] = w28_sb

            if has_bias:
                # biases (small / off critical path; on the gpsimd queue)
                b1_sb = bp.tile([128, E_LOCAL, KF], fp32)
                nc.gpsimd.dma_start(
                    out=b1_sb[:], in_=b1_d[:].rearrange("e p f -> p e f")
                )
                b2_sb = bp.tile([128, E_LOCAL, D], fp32)
                b2_ap = b2_d[:]
                b2_bc = bass.AP(
                    tensor=b2_ap.tensor,
                    offset=b2_ap.offset,
                    ap=[[0, 128]] + [list(a) for a in b2_ap.ap],
                )
                nc.gpsimd.dma_start(out=b2_sb[:], in_=b2_bc)

            # Layer-2 partial tiles cost a full 16x512-cycle pass no matter
            # how few tokens they hold. Pack the remainder tokens of 3-slot
            # windows into <=32-token column groups and run up to 4 groups
            # concurrently in one PE pass (column tiling, tile_position
            # derived automatically from the PSUM base partition).
            WINDOWS = [(0, 3), (3, 6)]  # slots 6-7 keep their partial tiles
            packed = set()
            win_pieces = {}
            for w0, w1e in WINDOWS:
                pieces = []  # (slot, ht_col0, m, rbase)
                rbase = sum(
                    -(-(caps[s] % 128) // 32) * 32
                    for ww0, ww1 in WINDOWS if (ww0, ww1) < (w0, w1e)
                    for s in range(ww0, ww1) if caps[s] % 128
                )
                for s in range(w0, min(w1e, E_LOCAL)):
                    rem = caps[s] % 128
                    if rem == 0 or caps[s] == 0:
                        continue
                    full = caps[s] // 128
                    off = 0
                    while off < rem:
                        m = min(32, rem - off)
                        pieces.append((s, full * 128 + off, m, rbase + off))
                        off += m
                    rbase += -(-rem // 32) * 32
                n_passes = -(-len(pieces) // 4)
                n_slots = len({p[0] for p in pieces})
                if pieces and n_passes < n_slots:
                    win_pieces[(w0, w1e)] = pieces
                    packed.update({p[0] for p in pieces})
            RTOT = sum(
                -(-(caps[s] % 128) // 32) * 32
                for w0, w1e in win_pieces
                for s in range(w0, min(w1e, E_LOCAL)) if caps[s] % 128
            )
            r_sb = rp.tile([128, KF, max(RTOT, 32)], bf16, name="r_sb") if win_pieces else None
            ht_sbs = {}

            def epilogue(y_sb, py, rows, e, r0=0):
                if has_bias:
                    nc.vector.tensor_add(
                        y_sb[r0:r0 + rows, :],
                        py[r0:r0 + rows, :],
                        b2_sb[r0:r0 + rows, e, :],
                    )
                else:
                    nc.vector.tensor_copy(
                        y_sb[r0:r0 + rows, :], py[r0:r0 + rows, :]
                    )

            def epilogue2(y_sb, y8_sb, py, py8, rows, e, d0, d1):
                # y = py + py8/_W28_SCALE (+ b2). A DVE op may read only one
                # PSUM operand, so the scalar engine (idle during the L2
                # phase) descales py8 into SBUF first.
                nc.scalar.activation(
                    out=y8_sb[:rows, d0:d1],
                    in_=py8[:rows, d0:d1],
                    func=mybir.ActivationFunctionType.Copy,
                    scale=1.0 / _W28_SCALE,
                )
                nc.vector.tensor_add(
                    y_sb[:rows, d0:d1],
                    y8_sb[:rows, d0:d1],
                    py[:rows, d0:d1],
                )
                if has_bias:
                    nc.vector.tensor_add(
                        y_sb[:rows, d0:d1],
                        y_sb[:rows, d0:d1],
                        b2_sb[:rows, e, d0:d1],
                    )

            def packed_pass(w0, w1e):
                pieces = win_pieces[(w0, w1e)]
                for i0 in range(0, len(pieces), 4):
                    grp = pieces[i0:i0 + 4]
                    py = psy.tile([128, D], fp32, tag="py")
                    for k in range(KF):
                        for gi, (s, hc0, m, rb) in enumerate(grp):
                            nc.tensor.matmul(
                                py[32 * gi:32 * gi + m, :],
                                lhsT=r_sb[:, k, rb:rb + m],
                                rhs=w2_sbs[s][:, k * D:(k + 1) * D],
                                start=(k == 0),
                                stop=(k == KF - 1),
                                tile_position=(0, 32 * gi),
                            )
                    y_sb = yp.tile([128, D], fp32, tag="ysb")
                    for gi, (s, hc0, m, rb) in enumerate(grp):
                        epilogue(y_sb, py, m, s, r0=32 * gi)
                        row0 = yoff[s] + (caps[s] // 128) * 128 + (hc0 - (caps[s] // 128) * 128)
                        nc.sync.dma_start(
                            out=y_d[row0:row0 + m, :],
                            in_=y_sb[32 * gi:32 * gi + m, :],
                        )

            for e in range(E_LOCAL):
                cap = caps[e]
                if cap == 0:
                    continue
                if e + 1 < E_LOCAL and caps[e + 1] > 0:
                    load_slot(e + 1)
                w1_sb, w2_sb, xt_sb = w1_sbs[e], w2_sbs[e], xt_sbs[e]

                def l1_lhsT(k, f):
                    if e == 0:
                        if f < ZB:
                            b0 = KD * C0 + f * KD * 128 + k * 128
                            return boot_sb[:, b0:b0 + 128]
                        return w1_sb[:, f - ZB, k, :]
                    return w1_sb[:, k, f * 128:(f + 1) * 128]

                def l1_rhs(ci, cs, k):
                    if e == 0:
                        if ci == 0:
                            return boot_sb[:, k * cs:(k + 1) * cs]
                        return xt_sb[:, k * cs:(k + 1) * cs]
                    xb = KD * sum(chunks[e][:ci])
                    return xt_sb[:, xb + k * cs:xb + (k + 1) * cs]

                # layer 1: HT[f-tile, tok] = gelu(w1_tile.T @ XT + b1)
                ht_sb = hp.tile([128, KF, CAPMAX], bf16, tag="ht")
                ht8_sb = h8p.tile([128, FP8_KT, CAP8], f8, tag="ht8")
                for f in range(KF):
                    for ci, cs in enumerate(chunks[e]):
                        c0 = sum(chunks[e][:ci])
                        ph = psh.tile([128, 512], fp32, tag="ph")
                        for k in range(KD):
                            nc.tensor.matmul(
                                ph[:, :cs],
                                lhsT=l1_lhsT(k, f),
                                rhs=l1_rhs(ci, cs, k),
                                start=(k == 0),
                                stop=(k == KD - 1),
                            )
                        nc.scalar.activation(
                            out=ht_sb[:, f, c0:c0 + cs],
                            in_=ph[:, :cs],
                            func=mybir.ActivationFunctionType.Gelu,
                            bias=(b1_sb[:, e, f:f + 1] if has_bias else 0.0),
                            scale=1.0,
                        )
                        if f < FP8_KT:
                            # fp8 copy of layer-2 k-tiles 0..FP8_KT-1 for the
                            # DoubleRow matmul (DVE cast, off critical path)
                            nc.vector.tensor_copy(
                                ht8_sb[:, f, c0:c0 + cs],
                                ht_sb[:, f, c0:c0 + cs],
                            )

                if e in packed:
                    rem = cap % 128
                    rb0 = None
                    for (s, hc0, m, rb) in [p for w in win_pieces.values() for p in w]:
                        if s == e:
                            rb0 = rb
                            break
                    nc.vector.tensor_copy(
                        r_sb[:, :, rb0:rb0 + rem],
                        ht_sb[:, :, (cap // 128) * 128:cap],
                    )

                # layer 2: Y[t-tile, :] = HT_tile.T @ w2 + b2. k-tiles
                # 0..FP8_KT-1 run as one fp8 DoubleRow matmul into py8
                # (scaled by _W28_SCALE); the rest accumulate in py.
                w28_sb = w28_sbs[e]
                NT = cap // 128 if e in packed else -(-cap // 128)
                # Tiles are processed in pairs: both fp8 DoubleRow matmuls
                # back-to-back, then both bf16 chains — halving the number of
                # PE weight-path perf-mode switches, which cost a pipeline
                # hiccup each.
                tiles = list(range(NT))
                groups = []
                t = 0
                while t < NT:
                    if t + 1 < NT and not (e == E_LOCAL - 1 and t + 1 == NT - 1):
                        groups.append([t, t + 1]); t += 2
                    else:
                        groups.append([t]); t += 1
                for grp in groups:
                  pys = {}
                  py8s = {}
                  for t in grp:
                    tt = min(128, cap - t * 128)
                    py8_t = psy8.tile([128, D], fp32, tag="py8")
                    py8s[t] = py8_t
                    nc.tensor.matmul(
                        py8s[t][:tt, :],
                        lhsT=ht8_sb[:, :, t * 128:t * 128 + tt],
                        rhs=w28_sb[:, :, :],
                        start=True,
                        stop=True,
                        perf_mode=mybir.MatmulPerfMode.DoubleRow,
                    )
                  for t in grp:
                    tt = min(128, cap - t * 128)
                    py = psy.tile([128, D], fp32, tag="py")
                    pys[t] = py
                    py8 = py8s[t]
                    last_tile = e == E_LOCAL - 1 and t == NT - 1
                    # Tail critical path: the very last tile runs in two
                    # column halves so the first half's PSUM->SBUF move and
                    # output DMA overlap the second half's matmuls, and the
                    # final DMAs are split across both HWDGE rings (each
                    # stripes packets over all 16 SDMA engines; the gpsimd
                    # software queue does not).
                    col_splits = [(0, 256), (256, 512)] if last_tile else [(0, D)]
                    for (d0, d1) in col_splits:
                        for k in range(FP8_KT, KF):
                            nc.tensor.matmul(
                                py[:tt, d0:d1],
                                lhsT=ht_sb[:, k, t * 128:t * 128 + tt],
                                rhs=w2_sb[:, k * D + d0:k * D + d1],
                                start=(k == FP8_KT),
                                stop=(k == KF - 1),
                            )
                    y_sb = yp.tile([128, D], fp32, tag="ysb")
                    y8_sb = yp.tile([128, D], fp32, tag="y8sb")
                    if last_tile:
                        # The first column half's PSUM->SBUF move runs while
                        # the second half's matmuls stream; after the second
                        # move, the output leaves as full-row DMAs (2 KB
                        # contiguous DRAM rows — column-sliced 1 KB strided
                        # writes retire ~3x slower) split across both rings.
                        row0 = yoff[e] + t * 128
                        epilogue2(y_sb, y8_sb, py, py8, tt, e, 0, 256)
                        epilogue2(y_sb, y8_sb, py, py8, tt, e, 256, 512)
                        h0 = (tt + 1) // 2
                        nc.sync.dma_start(
                            out=y_d[row0:row0 + h0, :],
                            in_=y_sb[:h0, :],
                        )
                        nc.scalar.dma_start(
                            out=y_d[row0 + h0:row0 + tt, :],
                            in_=y_sb[h0:tt, :],
                        )
                    else:
                        epilogue2(y_sb, y8_sb, py, py8, tt, e, 0, D)
                        nc.sync.dma_start(
                            out=y_d[yoff[e] + t * 128: yoff[e] + t * 128 + tt, :],
                            in_=y_sb[:tt, :],
                        )

                for (w0, w1e) in list(win_pieces):
                    if e == min(w1e, E_LOCAL) - 1:
                        packed_pass(w0, w1e)

    nc.compile()
    return nc


def _get_nc(caps, has_bias):
    key = (tuple(caps), has_bias)
    if key not in _nc_cache:
        _nc_cache[key] = _build_nc(tuple(caps), has_bias)
    return _nc_cache[key]


def kernel(**inputs):
    x = np.asarray(inputs["inputs"], dtype=np.float32)
    disp = np.asarray(inputs["dispatch_order"])
    w1 = np.asarray(inputs["w1"], dtype=np.float32)
    b1 = np.asarray(inputs["b1"], dtype=np.float32)
    w2 = np.asarray(inputs["w2"], dtype=np.float32)
    b2 = np.asarray(inputs["b2"], dtype=np.float32)

    B, S, Dd = x.shape
    assert Dd == D
    T = B * S
    xf = x.reshape(T, D)
    e = disp.astype(np.int64)
    has_bias = bool(np.any(b1) or np.any(b2))

    counts = np.bincount(e, minlength=NUM_EXPERTS)
    order = np.argsort(e, kind="stable")
    xs = xf[order]  # tokens grouped by expert, original order within expert
    offs = np.zeros(NUM_EXPERTS + 1, dtype=np.int64)
    np.cumsum(counts, out=offs[1:])

    # assign experts to (slot, core): slot j of core c gets the (8j+c)-th
    # most-loaded expert -> tight per-slot caps, balanced cores
    by_load = np.argsort(-counts, kind="stable")
    slot_expert = by_load.reshape(E_LOCAL, N_CORES)  # [slot, core] -> expert id
    caps = tuple(int(counts[slot_expert[j]].max()) for j in range(E_LOCAL))
    xoff, yoff = _slot_geometry(caps)
    chunks = [_chunk_list(caps[j], j) for j in range(E_LOCAL)]

    # weights in device layout (partition-major; slot-0 w1 is additionally
    # f-tile-major so progressive f-blocks are contiguous per partition)
    w1b = w1.astype(_BF16).reshape(NUM_EXPERTS, KD, 128, F)
    w1p = np.ascontiguousarray(
        w1b.transpose(0, 2, 1, 3).reshape(NUM_EXPERTS, 128, KD * F)
    )
    w1zp = np.ascontiguousarray(
        w1b.reshape(NUM_EXPERTS, KD, 128, KF, 128)
        .transpose(0, 2, 3, 1, 4).reshape(NUM_EXPERTS, 128, KF * KD * 128)
    )
    w2p = np.ascontiguousarray(
        w2.astype(_BF16).reshape(NUM_EXPERTS, KF, 128, D)
        .transpose(0, 2, 1, 3).reshape(NUM_EXPERTS, 128, KF * D)
    )
    # fp8 copy of w2 k-tiles 0..FP8_KT-1 (F rows 0:FP8_KT*128), pre-scaled
    w28p = np.ascontiguousarray(
        (w2[:, :FP8_KT * 128, :] * _W28_SCALE).astype(_F8)
        .reshape(NUM_EXPERTS, FP8_KT, 128, D)
        .transpose(0, 2, 1, 3).reshape(NUM_EXPERTS, 128, FP8_KT * D)
    )
    b1r = np.ascontiguousarray(
        b1.reshape(NUM_EXPERTS, KF, 128).transpose(0, 2, 1)
    )  # [E, 128, KF]
    xsb = xs.astype(_BF16)

    in_maps = []
    for c in range(N_CORES):
        eids = [int(slot_expert[j, c]) for j in range(E_LOCAL)]
        xt = np.zeros((128, KD * xoff[-1]), dtype=_BF16)
        for j, ei in enumerate(eids):
            cnt = int(counts[ei])
            cap = caps[j]
            if cnt:
                xe = xsb[offs[ei]:offs[ei + 1]]  # [cnt, D]
                base = KD * xoff[j]
                for ci, cs in enumerate(chunks[j]):
                    t0 = sum(chunks[j][:ci])
                    n = max(0, min(cs, cnt - t0))
                    if n == 0:
                        continue
                    xc = xe[t0:t0 + n]  # [n, D]
                    xtj = xc.T.reshape(KD, 128, n).transpose(1, 0, 2)
                    cb = base + KD * t0
                    for k in range(KD):
                        xt[:, cb + k * cs:cb + k * cs + n] = xtj[:, k, :]
        # boot transfer: slot-0 xt chunk 0 + slot-0 w1 f-tiles [0:ZB)
        ZB = _W1Z_BOOT_TILES
        C0 = chunks[0][0]
        boot = np.concatenate(
            [xt[:, KD * xoff[0]:KD * xoff[0] + KD * C0],
             w1zp[eids[0]][:, :ZB * KD * 128]], axis=1
        )
        m = {
            "xt": xt,
            "boot": np.ascontiguousarray(boot),
            "w1z": np.ascontiguousarray(w1zp[eids[0]][:, ZB * KD * 128:]),
            "w1r": np.ascontiguousarray(w1p[eids[1:]]),
            "w2": np.ascontiguousarray(w2p[eids]),
            "w28": np.ascontiguousarray(w28p[eids]),
        }
        if has_bias:
            m["b1"] = np.ascontiguousarray(b1r[eids])
            m["b2"] = np.ascontiguousarray(b2[eids])
        in_maps.append(m)

    nc = _get_nc(caps, has_bias)
    global _last_in_maps
    _last_in_maps = in_maps
    from concourse.bass_utils import run_bass_kernel_spmd

    res = run_bass_kernel_spmd(nc, in_maps, core_ids=list(range(N_CORES)))

    out_sorted = np.empty((T, D), dtype=np.float32)
    for c in range(N_CORES):
        y = res.results[c]["y"]
        for j in range(E_LOCAL):
            ei = int(slot_expert[j, c])
            cnt = int(counts[ei])
            if cnt:
                out_sorted[offs[ei]:offs[ei + 1]] = y[yoff[j]:yoff[j] + cnt]

    out = np.empty((T, D), dtype=np.float32)
    out[order] = out_sorted
    return out.reshape(B, S, D)


# revision 39
# speedup vs baseline: 1.1145x; 1.0039x over previous
"""MoE expert-parallel kernel for Trainium2 (8 NeuronCores).

Strategy:
  - Host: route tokens to experts (stable sort by dispatch_order). Experts are
    assigned to (core, slot) pairs by descending token count: slot j of core c
    gets the (8*j + c)-th most-loaded expert, so all cores see nearly identical
    work and slot j's capacity cap_j = max over cores of its count (tight).
  - Device (SPMD, 8 cores, 8 expert slots/core):
    per slot: HT = gelu(w1^T-tiled @ XT + b1) computed transposed [F, tokens],
    then Y = HT^T @ w2 + b2 [tokens, D]; bf16 operands, fp32 PSUM accumulation,
    with layer-2 k-tiles 0-1 as an fp8 DoubleRow matmul (see FP8_KT below).
  - Host: scatter per-expert outputs back to original token order.

Startup critical path: the first matmul needs only slot-0's xt and the first
f-tiles of slot-0's w1. Slot-0 w1 is stored f-tile-major ([128, KF, KD, 128])
so progressive f-blocks are contiguous per partition, and the early blocks go
on the Scalar HWDGE ring while xt chunks go on the Sync ring — the two DGEs
generate descriptors in parallel and neither queues behind the other.

Exit critical path: the final tile's y rows are split across the Sync and
Scalar HWDGE rings (both stripe packets over all 16 SDMA engines); the GpSimd
software queue is avoided (it lumps a whole transfer onto one engine).

No cross-core collectives: each core owns a disjoint set of experts, hence a
disjoint set of output token rows.
"""

import sys

import numpy as np
import ml_dtypes

for _p in ("/opt/trn_rl_repo",):
    if _p not in sys.path:
        sys.path.insert(0, _p)

_BF16 = ml_dtypes.bfloat16
_F8 = getattr(ml_dtypes, "float8_e4m3", ml_dtypes.float8_e4m3fn)

NUM_EXPERTS = 64
N_CORES = 8
E_LOCAL = NUM_EXPERTS // N_CORES  # 8 expert slots per core
D = 512
F = 2048
KD = D // 128   # 4 contraction tiles for layer 1
KF = F // 128   # 16 contraction tiles for layer 2

# Layer-2 k-tiles 0-1 run as one fp8 DoubleRow matmul (2x PE rate). The fp8
# quantization error on 1/8 of the contraction keeps the end-to-end max
# relative error at ~1.6e-2 (vs 3.5e-3 pure-bf16), under the 2e-2 budget.
# w2's fp8 copy is pre-scaled by _W28_SCALE (its values ~0.02 would land in
# e4m3's denormal range unscaled); the partial sum accumulates in a separate
# PSUM tile and is descaled by the scalar engine in the epilogue.
FP8_KT = 2
_W28_SCALE = 64.0

_nc_cache = {}


def _chunk_list(cap, e):
    """Layer-1 token chunks per slot (PSUM free dim <= 512 fp32).

    Balanced halves for cap > 512: a tiny trailing chunk would pay a full
    weight-load pass for a handful of columns.
    """
    if cap == 0:
        return []
    if cap <= 512:
        return [cap]
    h = (cap + 1) // 2
    return [h, cap - h]


def _slot_geometry(caps):
    """Per-slot column offsets for xt and row offsets for y."""
    xoff = [0]
    yoff = [0]
    for c in caps:
        xoff.append(xoff[-1] + c)
        yoff.append(yoff[-1] + (-(-c // 128)) * 128)
    return xoff, yoff


# Slot-0 startup: the first xt chunk and w1 f-tile 0 are fused into one
# "boot" transfer (~3.3 KB per-partition descriptors — big descriptors are
# what the SDMA engines sustain high rates on) issued first on the Sync
# ring, so a single early completion unblocks the first matmuls. Later
# f-blocks: [1:2) and [2:4) on the Scalar ring (slow spin-up but needed
# later), [4:8) and [8:16) on Sync behind the rest of xt.
_W1Z_BOOT_TILES = 1


def _build_nc(caps, has_bias):
    """Build + compile the SPMD Bass program for per-slot capacities `caps`."""
    import concourse.bacc as bacc
    import concourse.bass as bass
    import concourse.mybir as mybir
    import concourse.tile as tile

    fp32 = mybir.dt.float32
    bf16 = mybir.dt.bfloat16
    f8 = mybir.dt.float8e4
    alu = mybir.AluOpType

    xoff, yoff = _slot_geometry(caps)
    XCOLS = xoff[-1]
    YROWS = yoff[-1]
    CAPMAX = max(caps)
    # DoubleRow LDWEIGHTS requires the k-pair step to be a multiple of 16
    # (s3_lw dual-fp8 AP restriction), so the fp8 ht tile pads its per-k-tile
    # column capacity.
    CAP8 = -(-CAPMAX // 16) * 16
    chunks = [_chunk_list(caps[e], e) for e in range(E_LOCAL)]

    nc = bacc.Bacc("TRN2", target_bir_lowering=False, debug=False)

    # xt/w1z/w1r/w2 are partition-major: one contiguous run per partition per
    # transfer -> 128 large DMA descriptors instead of 512-2048 small ones.
    # xt is chunk-major within a slot: [chunk0: k0|k1|k2|k3, chunk1: ...] so a
    # chunk's worth of tokens is one contiguous transfer.
    C0 = chunks[0][0]
    ZB = _W1Z_BOOT_TILES
    BOOTC = KD * C0 + ZB * KD * 128
    xt_d = nc.dram_tensor("xt", [128, KD * XCOLS], bf16, kind="ExternalInput")
    boot_d = nc.dram_tensor("boot", [128, BOOTC], bf16, kind="ExternalInput")
    w1z_d = nc.dram_tensor(
        "w1z", [128, (KF - ZB) * KD * 128], bf16, kind="ExternalInput"
    )
    w1r_d = nc.dram_tensor(
        "w1r", [E_LOCAL - 1, 128, KD * F], bf16, kind="ExternalInput"
    )
    w2_d = nc.dram_tensor("w2", [E_LOCAL, 128, KF * D], bf16, kind="ExternalInput")
    w28_d = nc.dram_tensor(
        "w28", [E_LOCAL, 128, FP8_KT * D], f8, kind="ExternalInput"
    )
    if has_bias:
        b1_d = nc.dram_tensor("b1", [E_LOCAL, 128, KF], fp32, kind="ExternalInput")
        b2_d = nc.dram_tensor("b2", [E_LOCAL, D], fp32, kind="ExternalInput")
    y_d = nc.dram_tensor("y", [YROWS, D], fp32, kind="ExternalOutput")

    with tile.TileContext(nc) as tc:
        with (
            tc.tile_pool(name="w1zpool", bufs=1) as w1zp,
            tc.tile_pool(name="wpool", bufs=2) as wp,
            tc.tile_pool(name="w2pool", bufs=4) as w2p,
            tc.tile_pool(name="rpool", bufs=1) as rp,
            tc.tile_pool(name="xpool", bufs=2) as xp,
            tc.tile_pool(name="hpool", bufs=2) as hp,
            tc.tile_pool(name="h8pool", bufs=2) as h8p,
            tc.tile_pool(name="w28pool", bufs=2) as w28p,
            tc.tile_pool(name="ypool", bufs=4) as yp,
            tc.tile_pool(name="bias", bufs=1) as bp,
            tc.tile_pool(name="psh", bufs=3, space="PSUM") as psh,
            tc.tile_pool(name="psy", bufs=3, space="PSUM") as psy,
            tc.tile_pool(name="psy8", bufs=2, space="PSUM") as psy8,
        ):
            w1_sbs = [None] * E_LOCAL
            w2_sbs = [None] * E_LOCAL
            w28_sbs = [None] * E_LOCAL
            xt_sbs = [None] * E_LOCAL

            def chunk_col0(e, ci):
                # column offset of chunk ci inside slot e's xt block
                return KD * xoff[e] + KD * sum(chunks[e][:ci])

            # --- slot-0 critical startup loads -------------------------------
            # The DMA issue order below IS the delivery order per ring (FIFO
            # start + packet round-robin), arranged by first-use time.
            cap0 = caps[0]
            assert cap0 > 0 and len(chunks[0]) == 2
            boot_sb = w1zp.tile([128, BOOTC], bf16, name="boot")
            nc.sync.dma_start(out=boot_sb[:], in_=boot_d[:])
            w1z_sb = w1zp.tile([128, KF - ZB, KD, 128], bf16, name="w1z")
            w1z_v = w1z_sb.rearrange("p a b c -> p (a b c)")

            def w1z_block(t0, t1, eng):
                # f-tiles [t0:t1) of slot-0 w1; tiles 0..ZB-1 live in boot
                eng.dma_start(
                    out=w1z_v[:, (t0 - ZB) * KD * 128:(t1 - ZB) * KD * 128],
                    in_=w1z_d[:, (t0 - ZB) * KD * 128:(t1 - ZB) * KD * 128],
                )

            w1z_block(1, 2, nc.scalar)
            w1z_block(2, 4, nc.scalar)
            # xt slot-0 chunk 1 (chunk 0 came in boot)
            cs1 = chunks[0][1]
            xt0_sb = xp.tile([128, KD * cs1], bf16, tag="xt")
            nc.sync.dma_start(
                out=xt0_sb[:],
                in_=xt_d[:, KD * xoff[0] + KD * C0:KD * xoff[1]],
            )
            w1z_block(4, 8, nc.sync)
            w1z_block(8, 16, nc.sync)
            w2z_sb = w2p.tile([128, KF * D], bf16, tag="w2")
            nc.sync.dma_start(out=w2z_sb[:], in_=w2_d[0])
            w28z_sb = w28p.tile([128, FP8_KT, D], f8, tag="w28")
            nc.sync.dma_start(
                out=w28z_sb.rearrange("p a b -> p (a b)"), in_=w28_d[0]
            )
            xt_sbs[0], w1_sbs[0], w2_sbs[0] = xt0_sb, w1z_sb, w2z_sb
            w28_sbs[0] = w28z_sb

            def load_slot(e):
                # Sync HWDGE ring: FIFO start order + packet-level round-robin.
                cap = caps[e]
                xt_sb = xp.tile([128, KD * cap], bf16, tag="xt")
                nc.sync.dma_start(
                    out=xt_sb[:],
                    in_=xt_d[:, KD * xoff[e]:KD * xoff[e + 1]],
                )
                w1_sb = wp.tile([128, KD, F], bf16, tag="w1")
                nc.sync.dma_start(
                    out=w1_sb.rearrange("p k f -> p (k f)"),
                    in_=w1r_d[e - 1],
                )
                w2_sb = w2p.tile([128, KF * D], bf16, tag="w2")
                nc.sync.dma_start(out=w2_sb[:], in_=w2_d[e])
                w28_sb = w28p.tile([128, FP8_KT, D], f8, tag="w28")
                nc.sync.dma_start(
                    out=w28_sb.rearrange("p a b -> p (a b)"), in_=w28_d[e]
                )
                xt_sbs[e], w1_sbs[e], w2_sbs[e] = xt_sb, w1_sb, w2_sb
                w28_sbs[e] = w28_sb

            if has_bias:
                # biases (small / off critical path; on the gpsimd queue)
                b1_sb = bp.tile([128, E_LOCAL, KF], fp32)
                nc.gpsimd.dma_start(
                    out=b1_sb[:], in_=b1_d[:].rearrange("e p f -> p e f")
                )
                b2_sb = bp.tile([128, E_LOCAL, D], fp32)
                b2_ap = b2_d[:]
                b2_bc = bass.AP(
                    tensor=b2_ap.tensor,
                    offset=b2_ap.offset,
                    ap=[[0, 128]] + [list(a) for a in b2_ap.ap],
                )
                nc.gpsimd.dma_start(out=b2_sb[:], in_=b2_bc)

            # Layer-2 partial tiles cost a full 16x512-cycle pass no matter
            # how few tokens they hold. Pack the remainder tokens of 3-slot
            # windows into <=32-token column groups and run up to 4 groups
            # concurrently in one PE pass (column tiling, tile_position
            # derived automatically from the PSUM base partition).
            WINDOWS = [(0, 3), (3, 6)]  # slots 6-7 keep their partial tiles
            packed = set()
            win_pieces = {}
            for w0, w1e in WINDOWS:
                pieces = []  # (slot, ht_col0, m, rbase)
                rbase = sum(
                    -(-(caps[s] % 128) // 32) * 32
                    for ww0, ww1 in WINDOWS if (ww0, ww1) < (w0, w1e)
                    for s in range(ww0, ww1) if caps[s] % 128
                )
                for s in range(w0, min(w1e, E_LOCAL)):
                    rem = caps[s] % 128
                    if rem == 0 or caps[s] == 0:
                        continue
                    full = caps[s] // 128
                    off = 0
                    while off < rem:
                        m = min(32, rem - off)
                        pieces.append((s, full * 128 + off, m, rbase + off))
                        off += m
                    rbase += -(-rem // 32) * 32
                n_passes = -(-len(pieces) // 4)
                n_slots = len({p[0] for p in pieces})
                if pieces and n_passes < n_slots:
                    win_pieces[(w0, w1e)] = pieces
                    packed.update({p[0] for p in pieces})
            RTOT = sum(
                -(-(caps[s] % 128) // 32) * 32
                for w0, w1e in win_pieces
                for s in range(w0, min(w1e, E_LOCAL)) if caps[s] % 128
            )
            r_sb = rp.tile([128, KF, max(RTOT, 32)], bf16, name="r_sb") if win_pieces else None
            ht_sbs = {}

            def epilogue(y_sb, py, rows, e, r0=0):
                if has_bias:
                    nc.vector.tensor_add(
                        y_sb[r0:r0 + rows, :],
                        py[r0:r0 + rows, :],
                        b2_sb[r0:r0 + rows, e, :],
                    )
                else:
                    nc.vector.tensor_copy(
                        y_sb[r0:r0 + rows, :], py[r0:r0 + rows, :]
                    )

            def epilogue2(y_sb, y8_sb, py, py8, rows, e, d0, d1):
                # y = py + py8/_W28_SCALE (+ b2). A DVE op may read only one
                # PSUM operand, so the scalar engine (idle during the L2
                # phase) descales py8 into SBUF first.
                nc.scalar.activation(
                    out=y8_sb[:rows, d0:d1],
                    in_=py8[:rows, d0:d1],
                    func=mybir.ActivationFunctionType.Copy,
                    scale=1.0 / _W28_SCALE,
                )
                nc.vector.tensor_add(
                    y_sb[:rows, d0:d1],
                    y8_sb[:rows, d0:d1],
                    py[:rows, d0:d1],
                )
                if has_bias:
                    nc.vector.tensor_add(
                        y_sb[:rows, d0:d1],
                        y_sb[:rows, d0:d1],
                        b2_sb[:rows, e, d0:d1],
                    )

            def packed_pass(w0, w1e):
                pieces = win_pieces[(w0, w1e)]
                for i0 in range(0, len(pieces), 4):
                    grp = pieces[i0:i0 + 4]
                    py = psy.tile([128, D], fp32, tag="py")
                    for k in range(KF):
                        for gi, (s, hc0, m, rb) in enumerate(grp):
                            nc.tensor.matmul(
                                py[32 * gi:32 * gi + m, :],
                                lhsT=r_sb[:, k, rb:rb + m],
                                rhs=w2_sbs[s][:, k * D:(k + 1) * D],
                                start=(k == 0),
                                stop=(k == KF - 1),
                                tile_position=(0, 32 * gi),
                            )
                    y_sb = yp.tile([128, D], fp32, tag="ysb")
                    for gi, (s, hc0, m, rb) in enumerate(grp):
                        epilogue(y_sb, py, m, s, r0=32 * gi)
                        row0 = yoff[s] + (caps[s] // 128) * 128 + (hc0 - (caps[s] // 128) * 128)
                        nc.sync.dma_start(
                            out=y_d[row0:row0 + m, :],
                            in_=y_sb[32 * gi:32 * gi + m, :],
                        )

            for e in range(E_LOCAL):
                cap = caps[e]
                if cap == 0:
                    continue
                if e + 1 < E_LOCAL and caps[e + 1] > 0:
                    load_slot(e + 1)
                w1_sb, w2_sb, xt_sb = w1_sbs[e], w2_sbs[e], xt_sbs[e]

                def l1_lhsT(k, f):
                    if e == 0:
                        if f < ZB:
                            b0 = KD * C0 + f * KD * 128 + k * 128
                            return boot_sb[:, b0:b0 + 128]
                        return w1_sb[:, f - ZB, k, :]
                    return w1_sb[:, k, f * 128:(f + 1) * 128]

                def l1_rhs(ci, cs, k):
                    if e == 0:
                        if ci == 0:
                            return boot_sb[:, k * cs:(k + 1) * cs]
                        return xt_sb[:, k * cs:(k + 1) * cs]
                    xb = KD * sum(chunks[e][:ci])
                    return xt_sb[:, xb + k * cs:xb + (k + 1) * cs]

                # layer 1: HT[f-tile, tok] = gelu(w1_tile.T @ XT + b1).
                # Slot 0 walks f-tiles in pairs with the chunk loop outside
                # the pair: (f0,c0),(f1,c0),(f0,c1),(f1,c1),(f2,c0),... so
                # while xt chunk 1 is still in flight at startup, the PE has
                # two boot-/early-resident f-tiles of chunk-0 work to run.
                ht_sb = hp.tile([128, KF, CAPMAX], bf16, tag="ht")
                ht8_sb = h8p.tile([128, FP8_KT, CAP8], f8, tag="ht8")
                if e == 0:
                    pieces_l1 = [
                        (fp + f, ci)
                        for fp in range(0, KF, 2)
                        for ci in range(len(chunks[e]))
                        for f in range(2)
                    ]
                else:
                    pieces_l1 = [
                        (f, ci)
                        for f in range(KF)
                        for ci in range(len(chunks[e]))
                    ]
                for (f, ci) in pieces_l1:
                    if True:
                        cs = chunks[e][ci]
                        c0 = sum(chunks[e][:ci])
                        ph = psh.tile([128, 512], fp32, tag="ph")
                        for k in range(KD):
                            nc.tensor.matmul(
                                ph[:, :cs],
                                lhsT=l1_lhsT(k, f),
                                rhs=l1_rhs(ci, cs, k),
                                start=(k == 0),
                                stop=(k == KD - 1),
                            )
                        nc.scalar.activation(
                            out=ht_sb[:, f, c0:c0 + cs],
                            in_=ph[:, :cs],
                            func=mybir.ActivationFunctionType.Gelu,
                            bias=(b1_sb[:, e, f:f + 1] if has_bias else 0.0),
                            scale=1.0,
                        )
                        if f < FP8_KT:
                            # fp8 copy of layer-2 k-tiles 0..FP8_KT-1 for the
                            # DoubleRow matmul (DVE cast, off critical path)
                            nc.vector.tensor_copy(
                                ht8_sb[:, f, c0:c0 + cs],
                                ht_sb[:, f, c0:c0 + cs],
                            )

                if e in packed:
                    rem = cap % 128
                    rb0 = None
                    for (s, hc0, m, rb) in [p for w in win_pieces.values() for p in w]:
                        if s == e:
                            rb0 = rb
                            break
                    nc.vector.tensor_copy(
                        r_sb[:, :, rb0:rb0 + rem],
                        ht_sb[:, :, (cap // 128) * 128:cap],
                    )

                # layer 2: Y[t-tile, :] = HT_tile.T @ w2 + b2. k-tiles
                # 0..FP8_KT-1 run as one fp8 DoubleRow matmul into py8
                # (scaled by _W28_SCALE); the rest accumulate in py.
                w28_sb = w28_sbs[e]
                NT = cap // 128 if e in packed else -(-cap // 128)
                # Tiles are processed in pairs: both fp8 DoubleRow matmuls
                # back-to-back, then both bf16 chains — halving the number of
                # PE weight-path perf-mode switches, which cost a pipeline
                # hiccup each.
                tiles = list(range(NT))
                groups = []
                t = 0
                while t < NT:
                    if t + 1 < NT and not (e == E_LOCAL - 1 and t + 1 == NT - 1):
                        groups.append([t, t + 1]); t += 2
                    else:
                        groups.append([t]); t += 1
                for grp in groups:
                  pys = {}
                  py8s = {}
                  for t in grp:
                    tt = min(128, cap - t * 128)
                    py8_t = psy8.tile([128, D], fp32, tag="py8")
                    py8s[t] = py8_t
                    nc.tensor.matmul(
                        py8s[t][:tt, :],
                        lhsT=ht8_sb[:, :, t * 128:t * 128 + tt],
                        rhs=w28_sb[:, :, :],
                        start=True,
                        stop=True,
                        perf_mode=mybir.MatmulPerfMode.DoubleRow,
                    )
                  for t in grp:
                    tt = min(128, cap - t * 128)
                    py = psy.tile([128, D], fp32, tag="py")
                    pys[t] = py
                    py8 = py8s[t]
                    last_tile = e == E_LOCAL - 1 and t == NT - 1
                    # Tail critical path: the very last tile runs in two
                    # column halves so the first half's PSUM->SBUF move and
                    # output DMA overlap the second half's matmuls, and the
                    # final DMAs are split across both HWDGE rings (each
                    # stripes packets over all 16 SDMA engines; the gpsimd
                    # software queue does not).
                    col_splits = [(0, 256), (256, 512)] if last_tile else [(0, D)]
                    for (d0, d1) in col_splits:
                        for k in range(FP8_KT, KF):
                            nc.tensor.matmul(
                                py[:tt, d0:d1],
                                lhsT=ht_sb[:, k, t * 128:t * 128 + tt],
                                rhs=w2_sb[:, k * D + d0:k * D + d1],
                                start=(k == FP8_KT),
                                stop=(k == KF - 1),
                            )
                    y_sb = yp.tile([128, D], fp32, tag="ysb")
                    y8_sb = yp.tile([128, D], fp32, tag="y8sb")
                    if last_tile:
                        # The first column half's PSUM->SBUF move runs while
                        # the second half's matmuls stream; after the second
                        # move, the output leaves as full-row DMAs (2 KB
                        # contiguous DRAM rows — column-sliced 1 KB strided
                        # writes retire ~3x slower) split across both rings.
                        row0 = yoff[e] + t * 128
                        epilogue2(y_sb, y8_sb, py, py8, tt, e, 0, 256)
                        epilogue2(y_sb, y8_sb, py, py8, tt, e, 256, 512)
                        h0 = (tt + 1) // 2
                        nc.sync.dma_start(
                            out=y_d[row0:row0 + h0, :],
                            in_=y_sb[:h0, :],
                        )
                        nc.scalar.dma_start(
                            out=y_d[row0 + h0:row0 + tt, :],
                            in_=y_sb[h0:tt, :],
                        )
                    else:
                        epilogue2(y_sb, y8_sb, py, py8, tt, e, 0, D)
                        nc.sync.dma_start(
                            out=y_d[yoff[e] + t * 128: yoff[e] + t * 128 + tt, :],
                            in_=y_sb[:tt, :],
                        )

                for (w0, w1e) in list(win_pieces):
                    if e == min(w1e, E_LOCAL) - 1:
                        packed_pass(w0, w1e)

    nc.compile()
    return nc


def _get_nc(caps, has_bias):
    key = (tuple(caps), has_bias)
    if key not in _nc_cache:
        _nc_cache[key] = _build_nc(tuple(caps), has_bias)
    return _nc_cache[key]


def kernel(**inputs):
    x = np.asarray(inputs["inputs"], dtype=np.float32)
    disp = np.asarray(inputs["dispatch_order"])
    w1 = np.asarray(inputs["w1"], dtype=np.float32)
    b1 = np.asarray(inputs["b1"], dtype=np.float32)
    w2 = np.asarray(inputs["w2"], dtype=np.float32)
    b2 = np.asarray(inputs["b2"], dtype=np.float32)

    B, S, Dd = x.shape
    assert Dd == D
    T = B * S
    xf = x.reshape(T, D)
    e = disp.astype(np.int64)
    has_bias = bool(np.any(b1) or np.any(b2))

    counts = np.bincount(e, minlength=NUM_EXPERTS)
    order = np.argsort(e, kind="stable")
    xs = xf[order]  # tokens grouped by expert, original order within expert
    offs = np.zeros(NUM_EXPERTS + 1, dtype=np.int64)
    np.cumsum(counts, out=offs[1:])

    # assign experts to (slot, core): slot j of core c gets the (8j+c)-th
    # most-loaded expert -> tight per-slot caps, balanced cores
    by_load = np.argsort(-counts, kind="stable")
    slot_expert = by_load.reshape(E_LOCAL, N_CORES)  # [slot, core] -> expert id
    caps = tuple(int(counts[slot_expert[j]].max()) for j in range(E_LOCAL))
    xoff, yoff = _slot_geometry(caps)
    chunks = [_chunk_list(caps[j], j) for j in range(E_LOCAL)]

    # weights in device layout (partition-major; slot-0 w1 is additionally
    # f-tile-major so progressive f-blocks are contiguous per partition)
    w1b = w1.astype(_BF16).reshape(NUM_EXPERTS, KD, 128, F)
    w1p = np.ascontiguousarray(
        w1b.transpose(0, 2, 1, 3).reshape(NUM_EXPERTS, 128, KD * F)
    )
    w1zp = np.ascontiguousarray(
        w1b.reshape(NUM_EXPERTS, KD, 128, KF, 128)
        .transpose(0, 2, 3, 1, 4).reshape(NUM_EXPERTS, 128, KF * KD * 128)
    )
    w2p = np.ascontiguousarray(
        w2.astype(_BF16).reshape(NUM_EXPERTS, KF, 128, D)
        .transpose(0, 2, 1, 3).reshape(NUM_EXPERTS, 128, KF * D)
    )
    # fp8 copy of w2 k-tiles 0..FP8_KT-1 (F rows 0:FP8_KT*128), pre-scaled
    w28p = np.ascontiguousarray(
        (w2[:, :FP8_KT * 128, :] * _W28_SCALE).astype(_F8)
        .reshape(NUM_EXPERTS, FP8_KT, 128, D)
        .transpose(0, 2, 1, 3).reshape(NUM_EXPERTS, 128, FP8_KT * D)
    )
    b1r = np.ascontiguousarray(
        b1.reshape(NUM_EXPERTS, KF, 128).transpose(0, 2, 1)
    )  # [E, 128, KF]
    xsb = xs.astype(_BF16)

    in_maps = []
    for c in range(N_CORES):
        eids = [int(slot_expert[j, c]) for j in range(E_LOCAL)]
        xt = np.zeros((128, KD * xoff[-1]), dtype=_BF16)
        for j, ei in enumerate(eids):
            cnt = int(counts[ei])
            cap = caps[j]
            if cnt:
                xe = xsb[offs[ei]:offs[ei + 1]]  # [cnt, D]
                base = KD * xoff[j]
                for ci, cs in enumerate(chunks[j]):
                    t0 = sum(chunks[j][:ci])
                    n = max(0, min(cs, cnt - t0))
                    if n == 0:
                        continue
                    xc = xe[t0:t0 + n]  # [n, D]
                    xtj = xc.T.reshape(KD, 128, n).transpose(1, 0, 2)
                    cb = base + KD * t0
                    for k in range(KD):
                        xt[:, cb + k * cs:cb + k * cs + n] = xtj[:, k, :]
        # boot transfer: slot-0 xt chunk 0 + slot-0 w1 f-tiles [0:ZB)
        ZB = _W1Z_BOOT_TILES
        C0 = chunks[0][0]
        boot = np.concatenate(
            [xt[:, KD * xoff[0]:KD * xoff[0] + KD * C0],
             w1zp[eids[0]][:, :ZB * KD * 128]], axis=1
        )
        m = {
            "xt": xt,
            "boot": np.ascontiguousarray(boot),
            "w1z": np.ascontiguousarray(w1zp[eids[0]][:, ZB * KD * 128:]),
            "w1r": np.ascontiguousarray(w1p[eids[1:]]),
            "w2": np.ascontiguousarray(w2p[eids]),
            "w28": np.ascontiguousarray(w28p[eids]),
        }
        if has_bias:
            m["b1"] = np.ascontiguousarray(b1r[eids])
            m["b2"] = np.ascontiguousarray(b2[eids])
        in_maps.append(m)

    nc = _get_nc(caps, has_bias)
    global _last_in_maps
    _last_in_maps = in_maps
    from concourse.bass_utils import run_bass_kernel_spmd

    res = run_bass_kernel_spmd(nc, in_maps, core_ids=list(range(N_CORES)))

    out_sorted = np.empty((T, D), dtype=np.float32)
    for c in range(N_CORES):
        y = res.results[c]["y"]
        for j in range(E_LOCAL):
            ei = int(slot_expert[j, c])
            cnt = int(counts[ei])
            if cnt:
                out_sorted[offs[ei]:offs[ei + 1]] = y[yoff[j]:yoff[j] + cnt]

    out = np.empty((T, D), dtype=np.float32)
    out[order] = out_sorted
    return out.reshape(B, S, D)


# revision 40
# speedup vs baseline: 1.1156x; 1.0010x over previous
"""MoE expert-parallel kernel for Trainium2 (8 NeuronCores).

Strategy:
  - Host: route tokens to experts (stable sort by dispatch_order). Experts are
    assigned to (core, slot) pairs by descending token count: slot j of core c
    gets the (8*j + c)-th most-loaded expert, so all cores see nearly identical
    work and slot j's capacity cap_j = max over cores of its count (tight).
  - Device (SPMD, 8 cores, 8 expert slots/core):
    per slot: HT = gelu(w1^T-tiled @ XT + b1) computed transposed [F, tokens],
    then Y = HT^T @ w2 + b2 [tokens, D]; bf16 operands, fp32 PSUM accumulation,
    with layer-2 k-tiles 0-1 as an fp8 DoubleRow matmul (see FP8_KT below).
  - Host: scatter per-expert outputs back to original token order.

Startup critical path: the first matmul needs only slot-0's xt and the first
f-tiles of slot-0's w1. Slot-0 w1 is stored f-tile-major ([128, KF, KD, 128])
so progressive f-blocks are contiguous per partition, and the early blocks go
on the Scalar HWDGE ring while xt chunks go on the Sync ring — the two DGEs
generate descriptors in parallel and neither queues behind the other.

Exit critical path: the final tile's y rows are split across the Sync and
Scalar HWDGE rings (both stripe packets over all 16 SDMA engines); the GpSimd
software queue is avoided (it lumps a whole transfer onto one engine).

No cross-core collectives: each core owns a disjoint set of experts, hence a
disjoint set of output token rows.
"""

import sys

import numpy as np
import ml_dtypes

for _p in ("/opt/trn_rl_repo",):
    if _p not in sys.path:
        sys.path.insert(0, _p)

_BF16 = ml_dtypes.bfloat16
_F8 = getattr(ml_dtypes, "float8_e4m3", ml_dtypes.float8_e4m3fn)

NUM_EXPERTS = 64
N_CORES = 8
E_LOCAL = NUM_EXPERTS // N_CORES  # 8 expert slots per core
D = 512
F = 2048
KD = D // 128   # 4 contraction tiles for layer 1
KF = F // 128   # 16 contraction tiles for layer 2

# Layer-2 k-tiles 0-1 run as one fp8 DoubleRow matmul (2x PE rate). The fp8
# quantization error on 1/8 of the contraction keeps the end-to-end max
# relative error at ~1.6e-2 (vs 3.5e-3 pure-bf16), under the 2e-2 budget.
# w2's fp8 copy is pre-scaled by _W28_SCALE (its values ~0.02 would land in
# e4m3's denormal range unscaled); the partial sum accumulates in a separate
# PSUM tile and is descaled by the scalar engine in the epilogue.
FP8_KT = 2
_W28_SCALE = 64.0

_nc_cache = {}


def _chunk_list(cap, e):
    """Layer-1 token chunks per slot (PSUM free dim <= 512 fp32).

    Balanced halves for cap > 512: a tiny trailing chunk would pay a full
    weight-load pass for a handful of columns.
    """
    if cap == 0:
        return []
    if cap <= 512:
        return [cap]
    h = (cap + 1) // 2
    return [h, cap - h]


def _slot_geometry(caps):
    """Per-slot column offsets for xt and row offsets for y."""
    xoff = [0]
    yoff = [0]
    for c in caps:
        xoff.append(xoff[-1] + c)
        yoff.append(yoff[-1] + (-(-c // 128)) * 128)
    return xoff, yoff


# Slot-0 startup: the first xt chunk and w1 f-tile 0 are fused into one
# "boot" transfer (~3.3 KB per-partition descriptors — big descriptors are
# what the SDMA engines sustain high rates on) issued first on the Sync
# ring, so a single early completion unblocks the first matmuls. Later
# f-blocks: [1:2) and [2:4) on the Scalar ring (slow spin-up but needed
# later), [4:8) and [8:16) on Sync behind the rest of xt.
_W1Z_BOOT_TILES = 1


def _build_nc(caps, has_bias):
    """Build + compile the SPMD Bass program for per-slot capacities `caps`."""
    import concourse.bacc as bacc
    import concourse.bass as bass
    import concourse.mybir as mybir
    import concourse.tile as tile

    fp32 = mybir.dt.float32
    bf16 = mybir.dt.bfloat16
    f8 = mybir.dt.float8e4
    alu = mybir.AluOpType

    xoff, yoff = _slot_geometry(caps)
    XCOLS = xoff[-1]
    YROWS = yoff[-1]
    CAPMAX = max(caps)
    # DoubleRow LDWEIGHTS requires the k-pair step to be a multiple of 16
    # (s3_lw dual-fp8 AP restriction), so the fp8 ht tile pads its per-k-tile
    # column capacity.
    CAP8 = -(-CAPMAX // 16) * 16
    chunks = [_chunk_list(caps[e], e) for e in range(E_LOCAL)]

    nc = bacc.Bacc("TRN2", target_bir_lowering=False, debug=False)

    # xt/w1z/w1r/w2 are partition-major: one contiguous run per partition per
    # transfer -> 128 large DMA descriptors instead of 512-2048 small ones.
    # xt is chunk-major within a slot: [chunk0: k0|k1|k2|k3, chunk1: ...] so a
    # chunk's worth of tokens is one contiguous transfer.
    C0 = chunks[0][0]
    ZB = _W1Z_BOOT_TILES
    BOOTC = KD * C0 + ZB * KD * 128
    xt_d = nc.dram_tensor("xt", [128, KD * XCOLS], bf16, kind="ExternalInput")
    boot_d = nc.dram_tensor("boot", [128, BOOTC], bf16, kind="ExternalInput")
    w1z_d = nc.dram_tensor(
        "w1z", [128, (KF - ZB) * KD * 128], bf16, kind="ExternalInput"
    )
    w1r_d = nc.dram_tensor(
        "w1r", [E_LOCAL - 1, 128, KD * F], bf16, kind="ExternalInput"
    )
    w2_d = nc.dram_tensor("w2", [E_LOCAL, 128, KF * D], bf16, kind="ExternalInput")
    w28_d = nc.dram_tensor(
        "w28", [E_LOCAL, 128, FP8_KT * D], f8, kind="ExternalInput"
    )
    if has_bias:
        b1_d = nc.dram_tensor("b1", [E_LOCAL, 128, KF], fp32, kind="ExternalInput")
        b2_d = nc.dram_tensor("b2", [E_LOCAL, D], fp32, kind="ExternalInput")
    y_d = nc.dram_tensor("y", [YROWS, D], fp32, kind="ExternalOutput")

    with tile.TileContext(nc) as tc:
        with (
            tc.tile_pool(name="w1zpool", bufs=1) as w1zp,
            tc.tile_pool(name="wpool", bufs=2) as wp,
            tc.tile_pool(name="w2pool", bufs=4) as w2p,
            tc.tile_pool(name="rpool", bufs=1) as rp,
            tc.tile_pool(name="xpool", bufs=2) as xp,
            tc.tile_pool(name="hpool", bufs=2) as hp,
            tc.tile_pool(name="h8pool", bufs=2) as h8p,
            tc.tile_pool(name="w28pool", bufs=2) as w28p,
            tc.tile_pool(name="ypool", bufs=4) as yp,
            tc.tile_pool(name="bias", bufs=1) as bp,
            tc.tile_pool(name="psh", bufs=4, space="PSUM") as psh,
            tc.tile_pool(name="psy", bufs=2, space="PSUM") as psy,
            tc.tile_pool(name="psy8", bufs=2, space="PSUM") as psy8,
        ):
            w1_sbs = [None] * E_LOCAL
            w2_sbs = [None] * E_LOCAL
            w28_sbs = [None] * E_LOCAL
            xt_sbs = [None] * E_LOCAL

            def chunk_col0(e, ci):
                # column offset of chunk ci inside slot e's xt block
                return KD * xoff[e] + KD * sum(chunks[e][:ci])

            # --- slot-0 critical startup loads -------------------------------
            # The DMA issue order below IS the delivery order per ring (FIFO
            # start + packet round-robin), arranged by first-use time.
            cap0 = caps[0]
            assert cap0 > 0 and len(chunks[0]) == 2
            boot_sb = w1zp.tile([128, BOOTC], bf16, name="boot")
            nc.sync.dma_start(out=boot_sb[:], in_=boot_d[:])
            w1z_sb = w1zp.tile([128, KF - ZB, KD, 128], bf16, name="w1z")
            w1z_v = w1z_sb.rearrange("p a b c -> p (a b c)")

            def w1z_block(t0, t1, eng):
                # f-tiles [t0:t1) of slot-0 w1; tiles 0..ZB-1 live in boot
                eng.dma_start(
                    out=w1z_v[:, (t0 - ZB) * KD * 128:(t1 - ZB) * KD * 128],
                    in_=w1z_d[:, (t0 - ZB) * KD * 128:(t1 - ZB) * KD * 128],
                )

            w1z_block(1, 2, nc.scalar)
            w1z_block(2, 4, nc.scalar)
            # xt slot-0 chunk 1 (chunk 0 came in boot)
            cs1 = chunks[0][1]
            xt0_sb = xp.tile([128, KD * cs1], bf16, tag="xt")
            nc.sync.dma_start(
                out=xt0_sb[:],
                in_=xt_d[:, KD * xoff[0] + KD * C0:KD * xoff[1]],
            )
            w1z_block(4, 8, nc.sync)
            w1z_block(8, 16, nc.sync)
            w2z_sb = w2p.tile([128, KF * D], bf16, tag="w2")
            nc.sync.dma_start(out=w2z_sb[:], in_=w2_d[0])
            w28z_sb = w28p.tile([128, FP8_KT, D], f8, tag="w28")
            nc.sync.dma_start(
                out=w28z_sb.rearrange("p a b -> p (a b)"), in_=w28_d[0]
            )
            xt_sbs[0], w1_sbs[0], w2_sbs[0] = xt0_sb, w1z_sb, w2z_sb
            w28_sbs[0] = w28z_sb

            def load_slot(e):
                # Sync HWDGE ring: FIFO start order + packet-level round-robin.
                cap = caps[e]
                xt_sb = xp.tile([128, KD * cap], bf16, tag="xt")
                nc.sync.dma_start(
                    out=xt_sb[:],
                    in_=xt_d[:, KD * xoff[e]:KD * xoff[e + 1]],
                )
                w1_sb = wp.tile([128, KD, F], bf16, tag="w1")
                nc.sync.dma_start(
                    out=w1_sb.rearrange("p k f -> p (k f)"),
                    in_=w1r_d[e - 1],
                )
                w2_sb = w2p.tile([128, KF * D], bf16, tag="w2")
                nc.sync.dma_start(out=w2_sb[:], in_=w2_d[e])
                w28_sb = w28p.tile([128, FP8_KT, D], f8, tag="w28")
                nc.sync.dma_start(
                    out=w28_sb.rearrange("p a b -> p (a b)"), in_=w28_d[e]
                )
                xt_sbs[e], w1_sbs[e], w2_sbs[e] = xt_sb, w1_sb, w2_sb
                w28_sbs[e] = w28_sb

            if has_bias:
                # biases (small / off critical path; on the gpsimd queue)
                b1_sb = bp.tile([128, E_LOCAL, KF], fp32)
                nc.gpsimd.dma_start(
                    out=b1_sb[:], in_=b1_d[:].rearrange("e p f -> p e f")
                )
                b2_sb = bp.tile([128, E_LOCAL, D], fp32)
                b2_ap = b2_d[:]
                b2_bc = bass.AP(
                    tensor=b2_ap.tensor,
                    offset=b2_ap.offset,
                    ap=[[0, 128]] + [list(a) for a in b2_ap.ap],
                )
                nc.gpsimd.dma_start(out=b2_sb[:], in_=b2_bc)

            # Layer-2 partial tiles cost a full 16x512-cycle pass no matter
            # how few tokens they hold. Pack the remainder tokens of 3-slot
            # windows into <=32-token column groups and run up to 4 groups
            # concurrently in one PE pass (column tiling, tile_position
            # derived automatically from the PSUM base partition).
            WINDOWS = [(0, 3), (3, 6)]  # slots 6-7 keep their partial tiles
            packed = set()
            win_pieces = {}
            for w0, w1e in WINDOWS:
                pieces = []  # (slot, ht_col0, m, rbase)
                rbase = sum(
                    -(-(caps[s] % 128) // 32) * 32
                    for ww0, ww1 in WINDOWS if (ww0, ww1) < (w0, w1e)
                    for s in range(ww0, ww1) if caps[s] % 128
                )
                for s in range(w0, min(w1e, E_LOCAL)):
                    rem = caps[s] % 128
                    if rem == 0 or caps[s] == 0:
                        continue
                    full = caps[s] // 128
                    off = 0
                    while off < rem:
                        m = min(32, rem - off)
                        pieces.append((s, full * 128 + off, m, rbase + off))
                        off += m
                    rbase += -(-rem // 32) * 32
                n_passes = -(-len(pieces) // 4)
                n_slots = len({p[0] for p in pieces})
                if pieces and n_passes < n_slots:
                    win_pieces[(w0, w1e)] = pieces
                    packed.update({p[0] for p in pieces})
            RTOT = sum(
                -(-(caps[s] % 128) // 32) * 32
                for w0, w1e in win_pieces
                for s in range(w0, min(w1e, E_LOCAL)) if caps[s] % 128
            )
            r_sb = rp.tile([128, KF, max(RTOT, 32)], bf16, name="r_sb") if win_pieces else None
            ht_sbs = {}

            def epilogue(y_sb, py, rows, e, r0=0):
                if has_bias:
                    nc.vector.tensor_add(
                        y_sb[r0:r0 + rows, :],
                        py[r0:r0 + rows, :],
                        b2_sb[r0:r0 + rows, e, :],
                    )
                else:
                    nc.vector.tensor_copy(
                        y_sb[r0:r0 + rows, :], py[r0:r0 + rows, :]
                    )

            def epilogue2(y_sb, y8_sb, py, py8, rows, e, d0, d1):
                # y = py + py8/_W28_SCALE (+ b2). A DVE op may read only one
                # PSUM operand, so the scalar engine (idle during the L2
                # phase) descales py8 into SBUF first.
                nc.scalar.activation(
                    out=y8_sb[:rows, d0:d1],
                    in_=py8[:rows, d0:d1],
                    func=mybir.ActivationFunctionType.Copy,
                    scale=1.0 / _W28_SCALE,
                )
                nc.vector.tensor_add(
                    y_sb[:rows, d0:d1],
                    y8_sb[:rows, d0:d1],
                    py[:rows, d0:d1],
                )
                if has_bias:
                    nc.vector.tensor_add(
                        y_sb[:rows, d0:d1],
                        y_sb[:rows, d0:d1],
                        b2_sb[:rows, e, d0:d1],
                    )

            def packed_pass(w0, w1e):
                pieces = win_pieces[(w0, w1e)]
                for i0 in range(0, len(pieces), 4):
                    grp = pieces[i0:i0 + 4]
                    py = psy.tile([128, D], fp32, tag="py")
                    for k in range(KF):
                        for gi, (s, hc0, m, rb) in enumerate(grp):
                            nc.tensor.matmul(
                                py[32 * gi:32 * gi + m, :],
                                lhsT=r_sb[:, k, rb:rb + m],
                                rhs=w2_sbs[s][:, k * D:(k + 1) * D],
                                start=(k == 0),
                                stop=(k == KF - 1),
                                tile_position=(0, 32 * gi),
                            )
                    y_sb = yp.tile([128, D], fp32, tag="ysb")
                    for gi, (s, hc0, m, rb) in enumerate(grp):
                        epilogue(y_sb, py, m, s, r0=32 * gi)
                        row0 = yoff[s] + (caps[s] // 128) * 128 + (hc0 - (caps[s] // 128) * 128)
                        nc.sync.dma_start(
                            out=y_d[row0:row0 + m, :],
                            in_=y_sb[32 * gi:32 * gi + m, :],
                        )

            for e in range(E_LOCAL):
                cap = caps[e]
                if cap == 0:
                    continue
                if e + 1 < E_LOCAL and caps[e + 1] > 0:
                    load_slot(e + 1)
                w1_sb, w2_sb, xt_sb = w1_sbs[e], w2_sbs[e], xt_sbs[e]

                def l1_lhsT(k, f):
                    if e == 0:
                        if f < ZB:
                            b0 = KD * C0 + f * KD * 128 + k * 128
                            return boot_sb[:, b0:b0 + 128]
                        return w1_sb[:, f - ZB, k, :]
                    return w1_sb[:, k, f * 128:(f + 1) * 128]

                def l1_rhs(ci, cs, k):
                    if e == 0:
                        if ci == 0:
                            return boot_sb[:, k * cs:(k + 1) * cs]
                        return xt_sb[:, k * cs:(k + 1) * cs]
                    xb = KD * sum(chunks[e][:ci])
                    return xt_sb[:, xb + k * cs:xb + (k + 1) * cs]

                # layer 1: HT[f-tile, tok] = gelu(w1_tile.T @ XT + b1).
                # Slot 0 walks f-tiles in pairs with the chunk loop outside
                # the pair: (f0,c0),(f1,c0),(f0,c1),(f1,c1),(f2,c0),... so
                # while xt chunk 1 is still in flight at startup, the PE has
                # two boot-/early-resident f-tiles of chunk-0 work to run.
                ht_sb = hp.tile([128, KF, CAPMAX], bf16, tag="ht")
                ht8_sb = h8p.tile([128, FP8_KT, CAP8], f8, tag="ht8")
                if e == 0:
                    pieces_l1 = [
                        (fp + f, ci)
                        for fp in range(0, KF, 2)
                        for ci in range(len(chunks[e]))
                        for f in range(2)
                    ]
                else:
                    pieces_l1 = [
                        (f, ci)
                        for f in range(KF)
                        for ci in range(len(chunks[e]))
                    ]
                for (f, ci) in pieces_l1:
                    if True:
                        cs = chunks[e][ci]
                        c0 = sum(chunks[e][:ci])
                        ph = psh.tile([128, 512], fp32, tag="ph")
                        for k in range(KD):
                            nc.tensor.matmul(
                                ph[:, :cs],
                                lhsT=l1_lhsT(k, f),
                                rhs=l1_rhs(ci, cs, k),
                                start=(k == 0),
                                stop=(k == KD - 1),
                            )
                        nc.scalar.activation(
                            out=ht_sb[:, f, c0:c0 + cs],
                            in_=ph[:, :cs],
                            func=mybir.ActivationFunctionType.Gelu,
                            bias=(b1_sb[:, e, f:f + 1] if has_bias else 0.0),
                            scale=1.0,
                        )
                        if f < FP8_KT:
                            # fp8 copy of layer-2 k-tiles 0..FP8_KT-1 for the
                            # DoubleRow matmul (DVE cast, off critical path)
                            nc.vector.tensor_copy(
                                ht8_sb[:, f, c0:c0 + cs],
                                ht_sb[:, f, c0:c0 + cs],
                            )

                if e in packed:
                    rem = cap % 128
                    rb0 = None
                    for (s, hc0, m, rb) in [p for w in win_pieces.values() for p in w]:
                        if s == e:
                            rb0 = rb
                            break
                    nc.vector.tensor_copy(
                        r_sb[:, :, rb0:rb0 + rem],
                        ht_sb[:, :, (cap // 128) * 128:cap],
                    )

                # layer 2: Y[t-tile, :] = HT_tile.T @ w2 + b2. k-tiles
                # 0..FP8_KT-1 run as one fp8 DoubleRow matmul into py8
                # (scaled by _W28_SCALE); the rest accumulate in py.
                w28_sb = w28_sbs[e]
                NT = cap // 128 if e in packed else -(-cap // 128)
                # Tiles are processed in pairs: both fp8 DoubleRow matmuls
                # back-to-back, then both bf16 chains — halving the number of
                # PE weight-path perf-mode switches, which cost a pipeline
                # hiccup each.
                tiles = list(range(NT))
                groups = []
                t = 0
                while t < NT:
                    if t + 1 < NT and not (e == E_LOCAL - 1 and t + 1 == NT - 1):
                        groups.append([t, t + 1]); t += 2
                    else:
                        groups.append([t]); t += 1
                for grp in groups:
                  pys = {}
                  py8s = {}
                  for t in grp:
                    tt = min(128, cap - t * 128)
                    py8_t = psy8.tile([128, D], fp32, tag="py8")
                    py8s[t] = py8_t
                    nc.tensor.matmul(
                        py8s[t][:tt, :],
                        lhsT=ht8_sb[:, :, t * 128:t * 128 + tt],
                        rhs=w28_sb[:, :, :],
                        start=True,
                        stop=True,
                        perf_mode=mybir.MatmulPerfMode.DoubleRow,
                    )
                  for t in grp:
                    tt = min(128, cap - t * 128)
                    py = psy.tile([128, D], fp32, tag="py")
                    pys[t] = py
                    py8 = py8s[t]
                    last_tile = e == E_LOCAL - 1 and t == NT - 1
                    # Tail critical path: the very last tile runs in two
                    # column halves so the first half's PSUM->SBUF move and
                    # output DMA overlap the second half's matmuls, and the
                    # final DMAs are split across both HWDGE rings (each
                    # stripes packets over all 16 SDMA engines; the gpsimd
                    # software queue does not).
                    col_splits = [(0, 256), (256, 512)] if last_tile else [(0, D)]
                    for (d0, d1) in col_splits:
                        for k in range(FP8_KT, KF):
                            nc.tensor.matmul(
                                py[:tt, d0:d1],
                                lhsT=ht_sb[:, k, t * 128:t * 128 + tt],
                                rhs=w2_sb[:, k * D + d0:k * D + d1],
                                start=(k == FP8_KT),
                                stop=(k == KF - 1),
                            )
                    y_sb = yp.tile([128, D], fp32, tag="ysb")
                    y8_sb = yp.tile([128, D], fp32, tag="y8sb")
                    if last_tile:
                        # The first column half's PSUM->SBUF move runs while
                        # the second half's matmuls stream; after the second
                        # move, the output leaves as full-row DMAs (2 KB
                        # contiguous DRAM rows — column-sliced 1 KB strided
                        # writes retire ~3x slower) split across both rings.
                        row0 = yoff[e] + t * 128
                        epilogue2(y_sb, y8_sb, py, py8, tt, e, 0, 256)
                        epilogue2(y_sb, y8_sb, py, py8, tt, e, 256, 512)
                        h0 = (tt + 1) // 2
                        nc.sync.dma_start(
                            out=y_d[row0:row0 + h0, :],
                            in_=y_sb[:h0, :],
                        )
                        nc.scalar.dma_start(
                            out=y_d[row0 + h0:row0 + tt, :],
                            in_=y_sb[h0:tt, :],
                        )
                    else:
                        epilogue2(y_sb, y8_sb, py, py8, tt, e, 0, D)
                        nc.sync.dma_start(
                            out=y_d[yoff[e] + t * 128: yoff[e] + t * 128 + tt, :],
                            in_=y_sb[:tt, :],
                        )

                for (w0, w1e) in list(win_pieces):
                    if e == min(w1e, E_LOCAL) - 1:
                        packed_pass(w0, w1e)

    nc.compile()
    return nc


def _get_nc(caps, has_bias):
    key = (tuple(caps), has_bias)
    if key not in _nc_cache:
        _nc_cache[key] = _build_nc(tuple(caps), has_bias)
    return _nc_cache[key]


def kernel(**inputs):
    x = np.asarray(inputs["inputs"], dtype=np.float32)
    disp = np.asarray(inputs["dispatch_order"])
    w1 = np.asarray(inputs["w1"], dtype=np.float32)
    b1 = np.asarray(inputs["b1"], dtype=np.float32)
    w2 = np.asarray(inputs["w2"], dtype=np.float32)
    b2 = np.asarray(inputs["b2"], dtype=np.float32)

    B, S, Dd = x.shape
    assert Dd == D
    T = B * S
    xf = x.reshape(T, D)
    e = disp.astype(np.int64)
    has_bias = bool(np.any(b1) or np.any(b2))

    counts = np.bincount(e, minlength=NUM_EXPERTS)
    order = np.argsort(e, kind="stable")
    xs = xf[order]  # tokens grouped by expert, original order within expert
    offs = np.zeros(NUM_EXPERTS + 1, dtype=np.int64)
    np.cumsum(counts, out=offs[1:])

    # assign experts to (slot, core): slot j of core c gets the (8j+c)-th
    # most-loaded expert -> tight per-slot caps, balanced cores
    by_load = np.argsort(-counts, kind="stable")
    slot_expert = by_load.reshape(E_LOCAL, N_CORES)  # [slot, core] -> expert id
    caps = tuple(int(counts[slot_expert[j]].max()) for j in range(E_LOCAL))
    xoff, yoff = _slot_geometry(caps)
    chunks = [_chunk_list(caps[j], j) for j in range(E_LOCAL)]

    # weights in device layout (partition-major; slot-0 w1 is additionally
    # f-tile-major so progressive f-blocks are contiguous per partition)
    w1b = w1.astype(_BF16).reshape(NUM_EXPERTS, KD, 128, F)
    w1p = np.ascontiguousarray(
        w1b.transpose(0, 2, 1, 3).reshape(NUM_EXPERTS, 128, KD * F)
    )
    w1zp = np.ascontiguousarray(
        w1b.reshape(NUM_EXPERTS, KD, 128, KF, 128)
        .transpose(0, 2, 3, 1, 4).reshape(NUM_EXPERTS, 128, KF * KD * 128)
    )
    w2p = np.ascontiguousarray(
        w2.astype(_BF16).reshape(NUM_EXPERTS, KF, 128, D)
        .transpose(0, 2, 1, 3).reshape(NUM_EXPERTS, 128, KF * D)
    )
    # fp8 copy of w2 k-tiles 0..FP8_KT-1 (F rows 0:FP8_KT*128), pre-scaled
    w28p = np.ascontiguousarray(
        (w2[:, :FP8_KT * 128, :] * _W28_SCALE).astype(_F8)
        .reshape(NUM_EXPERTS, FP8_KT, 128, D)
        .transpose(0, 2, 1, 3).reshape(NUM_EXPERTS, 128, FP8_KT * D)
    )
    b1r = np.ascontiguousarray(
        b1.reshape(NUM_EXPERTS, KF, 128).transpose(0, 2, 1)
    )  # [E, 128, KF]
    xsb = xs.astype(_BF16)

    in_maps = []
    for c in range(N_CORES):
        eids = [int(slot_expert[j, c]) for j in range(E_LOCAL)]
        xt = np.zeros((128, KD * xoff[-1]), dtype=_BF16)
        for j, ei in enumerate(eids):
            cnt = int(counts[ei])
            cap = caps[j]
            if cnt:
                xe = xsb[offs[ei]:offs[ei + 1]]  # [cnt, D]
                base = KD * xoff[j]
                for ci, cs in enumerate(chunks[j]):
                    t0 = sum(chunks[j][:ci])
                    n = max(0, min(cs, cnt - t0))
                    if n == 0:
                        continue
                    xc = xe[t0:t0 + n]  # [n, D]
                    xtj = xc.T.reshape(KD, 128, n).transpose(1, 0, 2)
                    cb = base + KD * t0
                    for k in range(KD):
                        xt[:, cb + k * cs:cb + k * cs + n] = xtj[:, k, :]
        # boot transfer: slot-0 xt chunk 0 + slot-0 w1 f-tiles [0:ZB)
        ZB = _W1Z_BOOT_TILES
        C0 = chunks[0][0]
        boot = np.concatenate(
            [xt[:, KD * xoff[0]:KD * xoff[0] + KD * C0],
             w1zp[eids[0]][:, :ZB * KD * 128]], axis=1
        )
        m = {
            "xt": xt,
            "boot": np.ascontiguousarray(boot),
            "w1z": np.ascontiguousarray(w1zp[eids[0]][:, ZB * KD * 128:]),
            "w1r": np.ascontiguousarray(w1p[eids[1:]]),
            "w2": np.ascontiguousarray(w2p[eids]),
            "w28": np.ascontiguousarray(w28p[eids]),
        }
        if has_bias:
            m["b1"] = np.ascontiguousarray(b1r[eids])
            m["b2"] = np.ascontiguousarray(b2[eids])
        in_maps.append(m)

    nc = _get_nc(caps, has_bias)
    global _last_in_maps
    _last_in_maps = in_maps
    from concourse.bass_utils import run_bass_kernel_spmd

    res = run_bass_kernel_spmd(nc, in_maps, core_ids=list(range(N_CORES)))

    out_sorted = np.empty((T, D), dtype=np.float32)
    for c in range(N_CORES):
        y = res.results[c]["y"]
        for j in range(E_LOCAL):
            ei = int(slot_expert[j, c])
            cnt = int(counts[ei])
            if cnt:
                out_sorted[offs[ei]:offs[ei + 1]] = y[yoff[j]:yoff[j] + cnt]

    out = np.empty((T, D), dtype=np.float32)
    out[order] = out_sorted
    return out.reshape(B, S, D)
